# revision 1
# baseline (speedup 1.0000x reference)
"""Trainium2 Bass kernel for nn_AttentionBlock (GroupNorm + spatial self-attention + residual).

Full inputs in, full outputs out. Internally: data-parallel over the batch dim
(B=8) across 8 NeuronCores; each core runs an identical Bass/Tile program on
one [C=256, N=4096] image.

Per-core layout choices:
  - q,k stored [c, n] (c on partitions, 2 chunks of 128)
  - v stored transposed [n, c] (computed directly by swapping matmul operand
    roles, no on-device transpose pass)
  - attention scores computed transposed: S'[j,i] = (K^T Q)[j,i], j on
    partitions, so the AV contraction over j runs as PSUM-accumulated matmuls
  - softmax denominator: DVE accumulation of exp tiles over j-tiles, then a
    ones-vector matmul to reduce the 128 partitions; normalization applied to
    A via a broadcast tile (ones outer-product matmul)
  - all big matmuls in bf16 (1 cycle/row on PE vs 4 for fp32); accumulation is
    always fp32 in PSUM. exp(s/16) runs on ACT straight out of PSUM with the
    1/sqrt(C) folded into the activation scale; no max-subtraction (scores are
    in [-8, 7] for this distribution, exp is safe in fp32).
"""

import sys

try:
    import concourse  # noqa: F401
except ImportError:
    sys.path.insert(0, "/opt/trn_rl_repo")

import numpy as np
import ml_dtypes

import bass_rust as _bass_rust
import concourse.bacc as bacc
import concourse.tile as tile
from concourse import mybir
from concourse import bass_isa
from concourse.bass_utils import run_bass_kernel_spmd

F32 = mybir.dt.float32
BF16 = mybir.dt.bfloat16
AF = mybir.ActivationFunctionType
ALU = mybir.AluOpType
AX = mybir.AxisListType

# When True, q/k are stored fp8-e4m3 in DoubleRow [K,2,N] layout and the
# score matmuls run one fp8 DoubleRow matmul per (j-tile, i-stripe) at 2x PE
# rate (rel err ~6.7e-3 vs ~9e-4 for bf16; see numpy study).
S_FP8 = False

C = 256          # channels
N = 4096         # spatial positions (64*64)
GROUPS = 32      # groupnorm groups -> 8 channels per group
EPS = 1e-5
SCALE = C ** -0.5
NSTRIPE = 8      # stripes over the spatial dim
SW = N // NSTRIPE  # 512
NJT = N // 128   # 32 j-tiles
GSIZE = (C // GROUPS) * N  # elements per group = 32768


def _emit(nc, tc, d, parts="ABC"):
    """Emit the per-core program. d: dict of DRAM tensor handles."""
    const = tc.alloc_tile_pool(name="const", bufs=1)

    # --- x -> SBUF ---
    # The startup critical path (stats -> scale -> h -> everything) only needs
    # x at bf16 precision: bf16 quantization noise averages out over the
    # 32768-element group stats, and h is consumed in bf16 by the matmuls
    # anyway. So a half-size bf16 copy of x (host-prepared) lands first, and
    # the f32 x needed for the residual ~80us later arrives via the idle
    # gpsimd SWDGE path. bf16 DMAs are emitted before the weights (HWDGE
    # descriptor generation is a shared serial ~0.6us/DMA resource) and
    # spread across both HWDGE issuing engines.
    NSEG = 1
    SEG = N // NSEG
    xbf = []
    x_issuers = [nc.sync, nc.scalar, nc.scalar, nc.sync]
    for t in range(2):
        xb_ = const.tile([128, N], BF16, tag=f"xbf{t}", name=f"xbf{t}")
        for g in range(NSEG):
            x_issuers[t * NSEG + g].dma_start(
                xb_[:, g * SEG:(g + 1) * SEG],
                d["xbf"][t * 128:(t + 1) * 128, g * SEG:(g + 1) * SEG])
        xbf.append(xb_)

    # --- weights / params -> SBUF ---
    # All bf16 weights arrive as one packed [256, 1280] array (2 DMAs):
    # cols 0:768 = qkv_w.T, 768:1024 = proj_w.T, 1024:1280 (rows 0:128) =
    # v-bias broadcast tile. Small f32 params packed into [128, 10] (1 DMA).
    wb = []
    for t in range(2):
        w = const.tile([128, 1280], BF16, tag=f"wb{t}", name=f"wb{t}")
        nc.scalar.dma_start(w[:], d["wbig"][t * 128:(t + 1) * 128, :])
        wb.append(w)
    wq = [wb[0][:, 0:768], wb[1][:, 0:768]]
    wp = [wb[0][:, 768:1024], wb[1][:, 768:1024]]
    vbb = wb[0][:, 1024:1280]
    smalls = const.tile([128, 10], F32, tag="smalls")
    nc.scalar.dma_start(smalls[:], d["smalls"][:])
    qkvb = smalls[:, 0:4]
    projb = smalls[:, 4:6]
    nw = smalls[:, 6:8]
    nb = smalls[:, 8:10]
    gm = const.tile([128, 128], F32, tag="gm")
    nc.scalar.dma_start(gm[:], d["gm"][:])

    # f32 x tiles for the residual — allocated here, but their DMAs are
    # emitted after phase A so the (in-order) DMA resources service the
    # startup-critical bf16 x and weights first.
    xt = [const.tile([128, N], F32, tag=f"x{t}", name=f"x{t}") for t in range(2)]

    # --- phase A: groupnorm stats -> per-channel scale/bias ---
    # Fully per-chunk (chunk-major layout): chunk 0's scale/bias — and with
    # them the first h tiles and qkv matmuls — are ready before chunk 1's
    # stats have even landed.
    # pstats col for (t, kind, g) = (2t+kind)*NSEG + g; stats col = 2t+kind.
    pstats = const.tile([128, 4 * NSEG], F32, tag="pstats")
    stats = const.tile([128, 4], F32, tag="stats")
    scl = const.tile([128, 2], F32, tag="scl")
    bia = const.tile([128, 2], F32, tag="bia")
    gstats_mm = None
    with tc.tile_pool(name="scratch", bufs=2) as scr, \
         tc.tile_pool(name="pa_ps", bufs=1, space="PSUM") as pa_ps:
        for t in range(2):
            for g in range(NSEG):
                seg = xbf[t][:, g * SEG:(g + 1) * SEG]
                c0 = (2 * t + 0) * NSEG + g
                c1 = (2 * t + 1) * NSEG + g
                nc.vector.reduce_sum(pstats[:, c0:c0 + 1], seg, axis=AX.X)
                sq = scr.tile([128, SEG], F32, tag="sq")
                nc.scalar.activation(sq[:], seg, AF.Square,
                                     accum_out=pstats[:, c1:c1 + 1])
            for kind in range(2):
                tk = 2 * t + kind
                nc.vector.reduce_sum(stats[:, tk:tk + 1],
                                     pstats[:, tk * NSEG:(tk + 1) * NSEG], axis=AX.X)
            gstats = pa_ps.tile([128, 2], F32, tag=f"gstats{t}", name=f"gstats{t}")
            gstats_mm = nc.tensor.matmul(gstats[:], gm[:], stats[:, 2 * t:2 * t + 2],
                                         start=True, stop=True)
            # mex cols = [mean, ex2] for this chunk
            mex = const.tile([128, 2], F32, tag=f"mex{t}", name=f"mex{t}")
            nc.vector.tensor_scalar_mul(mex[:], gstats[:], 1.0 / GSIZE)
            mean = mex[:, 0:1]
            ex2 = mex[:, 1:2]
            var = const.tile([128, 1], F32, tag=f"var{t}", name=f"var{t}")
            std = const.tile([128, 1], F32, tag=f"std{t}", name=f"std{t}")
            rstd = const.tile([128, 1], F32, tag=f"rstd{t}", name=f"rstd{t}")
            negm2 = const.tile([128, 1], F32, tag=f"negm2{t}", name=f"negm2{t}")
            nc.vector.scalar_tensor_tensor(negm2[:], mean, -1.0, mean,
                                           op0=ALU.mult, op1=ALU.mult)
            nc.vector.scalar_tensor_tensor(var[:], ex2, EPS, negm2[:],
                                           op0=ALU.add, op1=ALU.add)
            nc.scalar.activation(std[:], var[:], AF.Sqrt)
            nc.vector.reciprocal(rstd[:], std[:])
            nc.vector.tensor_mul(scl[:, t:t + 1], nw[:, t:t + 1], rstd[:])
            mscl = const.tile([128, 1], F32, tag=f"mscl{t}", name=f"mscl{t}")
            nc.vector.tensor_mul(mscl[:], mean, scl[:, t:t + 1])
            nc.vector.tensor_sub(bia[:, t:t + 1], nb[:, t:t + 1], mscl[:])

    # f32 x for the residual — needed from the first stripe tail (~90us in);
    # issued via the idle gpsimd SWDGE path. The explicit dep on the stats
    # matmul keeps its transfers off the (in-order) DMA resources until the
    # startup-critical bf16-x/weights burst is done.
    for t in range(2):
        xdma = nc.gpsimd.dma_start(xt[t][:], d["x"][t * 128:(t + 1) * 128, :])
        _bass_rust.add_dep_helper(xdma.ins, gstats_mm.ins,
                                  reason="delay f32-x past startup DMA burst")

    # --- phase B: h = x*scl+bia (bf16), q,k ([c,n]) and vT ([n,c]) ---
    FP8 = mybir.dt.float8e4
    if S_FP8:
        # q/k in DoubleRow layout: partition p, element e <-> channel e*128+p
        qf8 = const.tile([128, 2, N], FP8, tag="qf8")
        kf8 = const.tile([128, 2, N], FP8, tag="kf8")
        qk = [qf8[:, 0, :], qf8[:, 1, :], kf8[:, 0, :], kf8[:, 1, :]]
    else:
        qk = []
        for i in range(4):  # q0,q1,k0,k1
            t_ = const.tile([128, N], BF16, tag=f"qk{i}", name=f"qk{i}")
            qk.append(t_)
    vt = []
    for j in range(NJT):
        t_ = const.tile([128, 256], BF16, tag=f"vt{j}", name=f"vt{j}")
        vt.append(t_)

    with tc.tile_pool(name="hpool", bufs=1) as hp, \
         tc.tile_pool(name="pb_ps", bufs=3, space="PSUM") as pbp, \
         tc.tile_pool(name="pv_ps", bufs=3, space="PSUM") as pvp:
        # h = x*scl + bia on ACT (per-partition scale/bias APs); pre-emit all
        # 16 tiles so production runs ahead of PE consumption.
        hs = []
        for s in range(NSTRIPE):
            sl = slice(s * SW, (s + 1) * SW)
            hts = []
            for t in range(2):
                ht = hp.tile([128, SW], BF16, tag=f"h{t}_{s}", name=f"h{t}_{s}")
                nc.scalar.activation(ht[:], xbf[t][:, sl], AF.Identity,
                                     bias=bia[:, t:t + 1], scale=scl[:, t:t + 1])
                hts.append(ht)
            hs.append(hts)
        for s in range(NSTRIPE):
            sl = slice(s * SW, (s + 1) * SW)
            hts = hs[s]
            for dt in (2, 3, 0, 1):  # k first: phase C's first matmuls need k
                ps = pbp.tile([128, SW], F32, tag="qkps", name="qkps")
                nc.tensor.matmul(ps[:], wq[0][:, dt * 128:(dt + 1) * 128], hts[0][:],
                                 start=True, stop=False)
                nc.tensor.matmul(ps[:], wq[1][:, dt * 128:(dt + 1) * 128], hts[1][:],
                                 start=False, stop=True)
                # split bias-copies q->DVE, k->ACT so neither engine exceeds
                # PE's ~27us in phase B (ACT also produces the h tiles)
                if dt < 2:
                    nc.vector.tensor_scalar_add(qk[dt][:, sl], ps[:], qkvb[:, dt:dt + 1])
                else:
                    nc.scalar.activation(qk[dt][:, sl], ps[:], AF.Identity,
                                         bias=qkvb[:, dt:dt + 1])
            for n4 in range(4):
                jt = s * 4 + n4
                psv = pvp.tile([128, 256], F32, tag="vtps", name="vtps")
                nc.tensor.matmul(psv[:], hts[0][:, n4 * 128:(n4 + 1) * 128],
                                 wq[0][:, 512:768], start=True, stop=False)
                nc.tensor.matmul(psv[:], hts[1][:, n4 * 128:(n4 + 1) * 128],
                                 wq[1][:, 512:768], start=False, stop=True)
                nc.vector.tensor_add(vt[jt][:], psv[:], vbb[:])

    # --- phase C: attention + proj + residual, per i-stripe ---
    if "C" not in parts:
        # timing variant: still write something to out so nothing is elided
        dummy = const.tile([128, 16], F32, tag="dummy")
        nc.vector.tensor_copy(dummy[:], xt[0][:, 0:16])
        nc.gpsimd.dma_start(d["out"][0:128, 0:16], dummy[:])
        const.release()
        return
    LAG = 6
    with tc.tile_pool(name="wpool", bufs=LAG + 3) as wpo, \
         tc.tile_pool(name="raccp", bufs=3) as rp, \
         tc.tile_pool(name="misc", bufs=2) as mp, \
         tc.tile_pool(name="s_ps", bufs=3, space="PSUM") as sp, \
         tc.tile_pool(name="a_ps", bufs=4, space="PSUM") as apo, \
         tc.tile_pool(name="o_ps", bufs=1, space="PSUM") as opo:

        def make_tail(ist, racc, a_ps):
            """Normalization + proj + residual for a finished stripe, split in
            three parts that are interleaved into the next stripe's matmul
            stream (the serial rsum->recip->mul chain hides behind PE work
            instead of stalling it)."""
            sl = slice(ist * SW, (ist + 1) * SW)
            st = {}

            def part1():
                # all-reduce over partitions on the (idle) gpsimd engine:
                # every partition ends up holding the softmax denominator row
                rall = mp.tile([128, 2 * SW], F32, tag="rall")
                nc.gpsimd.partition_all_reduce(rall[:], racc[:], 128,
                                               bass_isa.ReduceOp.add)
                st["rall"] = rall

            def part2a():
                rall = st["rall"]
                rsum = mp.tile([128, SW], F32, tag="rsum")
                nc.vector.tensor_add(rsum[:], rall[:, 0:SW], rall[:, SW:2 * SW])
                rinv = mp.tile([128, SW], F32, tag="rinv")
                nc.vector.reciprocal(rinv[:], rsum[:])
                st["rinv"] = rinv

            def part2b():
                a_sb = []
                for ct in range(2):
                    t_ = mp.tile([128, SW], BF16, tag=f"asb{ct}", name=f"asb{ct}")
                    nc.vector.tensor_mul(t_[:], a_ps[ct][:], st["rinv"][:])
                    a_sb.append(t_)
                st["a_sb"] = a_sb

            def part2():
                part2a()
                part2b()

            def part3():
                a_sb = st["a_sb"]
                for dt in range(2):
                    o_ps = opo.tile([128, SW], F32, tag="ops", name="ops")
                    nc.tensor.matmul(o_ps[:], wp[0][:, dt * 128:(dt + 1) * 128], a_sb[0][:],
                                     start=True, stop=False)
                    nc.tensor.matmul(o_ps[:], wp[1][:, dt * 128:(dt + 1) * 128], a_sb[1][:],
                                     start=False, stop=True)
                    o_sb = mp.tile([128, SW], F32, tag=f"osb{dt}", name=f"osb{dt}")
                    nc.vector.scalar_tensor_tensor(o_sb[:], o_ps[:], projb[:, dt:dt + 1],
                                                   xt[dt][:, sl], op0=ALU.add, op1=ALU.add)
                    nc.gpsimd.dma_start(d["out"][dt * 128:(dt + 1) * 128, sl], o_sb[:])

            return [part1, part2, part3, part2a, part2b]

        pending = None
        NPAIR = NJT // 2
        PLAG = LAG // 2
        for ist in range(NSTRIPE):
            sl = slice(ist * SW, (ist + 1) * SW)
            racc = rp.tile([128, 2 * SW], F32, tag="racc")
            a_ps = [apo.tile([128, SW], F32, tag="aps", name="aps") for _ in range(2)]
            # exp output halves of two consecutive j-tiles share one SBUF
            # tile, so the racc accumulation runs at [128,1024] granularity
            # (half the DVE per-op overhead) while PSUM stays per-jt
            # single-bank. AV matmuls run LAG steps behind production so the
            # (in-order) PE queue never head-of-line blocks on exp.
            w_pairs = {}
            for jt in range(NJT + LAG):
                if jt < NJT:
                    s_ps = sp.tile([128, SW], F32, tag="sps", name="sps")
                    if S_FP8:
                        nc.tensor.matmul(s_ps[:], kf8[:, :, jt * 128:(jt + 1) * 128],
                                         qf8[:, :, sl], start=True, stop=True,
                                         perf_mode=mybir.MatmulPerfMode.DoubleRow)
                    else:
                        nc.tensor.matmul(s_ps[:], qk[2][:, jt * 128:(jt + 1) * 128],
                                         qk[0][:, sl], start=True, stop=False)
                        nc.tensor.matmul(s_ps[:], qk[3][:, jt * 128:(jt + 1) * 128],
                                         qk[1][:, sl], start=False, stop=True)
                    p = jt // 2
                    if jt % 2 == 0:
                        w_pairs[p] = wpo.tile([128, 2 * SW], BF16, tag="wsb", name="wsb")
                    hsl = slice((jt % 2) * SW, (jt % 2 + 1) * SW)
                    nc.scalar.activation(w_pairs[p][:, hsl], s_ps[:], AF.Exp, scale=SCALE)
                    if jt % 2 == 1:
                        if p == 0:
                            nc.vector.tensor_copy(racc[:], w_pairs[p][:])
                        else:
                            nc.vector.tensor_add(racc[:], racc[:], w_pairs[p][:])
                if pending is not None:
                    if jt == 1:
                        pending[0]()
                    elif jt == 3:
                        pending[1]()
                    elif jt == 7:
                        pending[2]()
                        pending = None
                if ist == NSTRIPE - 1 and "noav" not in parts:
                    # last stripe: run the all-reduce and the fold/recip while
                    # the trailing AV matmuls still execute; only the a_sb
                    # muls and proj remain after the loop.
                    if jt == NJT:
                        last_tail = make_tail(ist, racc, a_ps)
                        last_tail[0]()          # part1: all-reduce
                        pending = None
                    elif jt == NJT + 3:
                        last_tail[3]()          # part2a: fold + reciprocal
                        pending = [last_tail[4], last_tail[2]]  # muls, proj
                if "noav" in parts:
                    continue
                if jt >= LAG:
                    j2 = jt - LAG
                    w2 = w_pairs[j2 // 2]
                    if j2 % 2 == 1:
                        del w_pairs[j2 // 2]
                    hsl = slice((j2 % 2) * SW, (j2 % 2 + 1) * SW)
                    for ct in range(2):
                        nc.tensor.matmul(a_ps[ct][:], vt[j2][:, ct * 128:(ct + 1) * 128],
                                         w2[:, hsl], start=(j2 == 0), stop=(j2 == NJT - 1))
            if "noav" in parts:
                o_sb = mp.tile([128, SW], F32, tag="osb0", name="osb0")
                nc.vector.tensor_add(o_sb[:], racc[:, 0:SW], xt[0][:, sl])
                nc.gpsimd.dma_start(d["out"][0:128, sl], o_sb[:])
                continue
            if ist < NSTRIPE - 1:
                pending = make_tail(ist, racc, a_ps)
        if pending is not None:
            for p in pending:
                p()

    const.release()


def build_program(repeat: int = 1, parts: str = "ABC"):
    nc = bacc.Bacc("TRN2", target_bir_lowering=False, debug=False, num_devices=8)
    d = {
        "x": nc.declare_dram_parameter("x", [C, N], F32, isOutput=False),
        "xbf": nc.declare_dram_parameter("xbf", [C, N], BF16, isOutput=False),
        "wbig": nc.declare_dram_parameter("wbig", [C, 1280], BF16, isOutput=False),
        "smalls": nc.declare_dram_parameter("smalls", [128, 10], F32, isOutput=False),
        "gm": nc.declare_dram_parameter("gm", [128, 128], F32, isOutput=False),
        "out": nc.declare_dram_parameter("out", [C, N], F32, isOutput=True),
    }
    with tile.TileContext(nc) as tc:
        for _ in range(repeat):
            _emit(nc, tc, d, parts)
    nc.compile()
    return nc


def make_in_maps(x, norm_w, norm_b, qkv_w, qkv_b, proj_w, proj_b):
    x = np.asarray(x, np.float32)
    B = x.shape[0]
    qkv_w = np.asarray(qkv_w, np.float32)
    qkv_b = np.asarray(qkv_b, np.float32)
    proj_w = np.asarray(proj_w, np.float32)
    proj_b = np.asarray(proj_b, np.float32)
    wbig = np.zeros((256, 1280), np.float32)
    wbig[:, 0:768] = qkv_w.T
    wbig[:, 768:1024] = proj_w.T
    wbig[0:128, 1024:1280] = np.tile(qkv_b[512:].reshape(1, 256), (128, 1))
    smalls = np.zeros((128, 10), np.float32)
    smalls[:, 0:4] = qkv_b[:512].reshape(4, 128).T
    smalls[:, 4:6] = proj_b.reshape(2, 128).T
    smalls[:, 6:8] = np.asarray(norm_w, np.float32).reshape(2, 128).T
    smalls[:, 8:10] = np.asarray(norm_b, np.float32).reshape(2, 128).T
    shared = {
        "wbig": wbig.astype(ml_dtypes.bfloat16),
        "smalls": smalls,
        "gm": (np.arange(128)[:, None] // 8 == np.arange(128)[None, :] // 8).astype(np.float32),
    }
    return [
        dict(shared,
             x=np.ascontiguousarray(x[b].reshape(C, N)),
             xbf=np.ascontiguousarray(x[b].reshape(C, N)).astype(ml_dtypes.bfloat16))
        for b in range(B)
    ]


_NC_CACHE = {}


def get_program(repeat: int = 1):
    if repeat not in _NC_CACHE:
        _NC_CACHE[repeat] = build_program(repeat)
    return _NC_CACHE[repeat]


def kernel(x, norm_w, norm_b, qkv_w, qkv_b, proj_w, proj_b):
    x = np.asarray(x, np.float32)
    B, C_, H_, W_ = x.shape
    in_maps = make_in_maps(x, norm_w, norm_b, qkv_w, qkv_b, proj_w, proj_b)
    nc = get_program()
    res = run_bass_kernel_spmd(nc, in_maps, core_ids=list(range(len(in_maps))))
    out = np.stack([np.asarray(res.results[b]["out"], np.float32) for b in range(B)])
    return out.reshape(B, C_, H_, W_)



# revision 2
# speedup vs baseline: 1.3179x; 1.3179x over previous
"""Trainium2 Bass kernel for nn_AttentionBlock (GroupNorm + spatial self-attention + residual).

Full inputs in, full outputs out. Internally: data-parallel over the batch dim
(B=8) across 8 NeuronCores; each core runs an identical Bass/Tile program on
one [C=256, N=4096] image.

Per-core layout choices:
  - q,k stored [c, n] (c on partitions, 2 chunks of 128)
  - v stored transposed [n, c] (computed directly by swapping matmul operand
    roles, no on-device transpose pass)
  - attention scores computed transposed: S'[j,i] = (K^T Q)[j,i], j on
    partitions, so the AV contraction over j runs as PSUM-accumulated matmuls
  - softmax denominator: DVE accumulation of exp tiles over j-tiles, then a
    ones-vector matmul to reduce the 128 partitions; normalization applied to
    A via a broadcast tile (ones outer-product matmul)
  - all big matmuls in bf16 (1 cycle/row on PE vs 4 for fp32); accumulation is
    always fp32 in PSUM. exp(s/16) runs on ACT straight out of PSUM with the
    1/sqrt(C) folded into the activation scale; no max-subtraction (scores are
    in [-8, 7] for this distribution, exp is safe in fp32).
"""

import sys

try:
    import concourse  # noqa: F401
except ImportError:
    sys.path.insert(0, "/opt/trn_rl_repo")

import numpy as np
import ml_dtypes

import bass_rust as _bass_rust
import concourse.bacc as bacc
import concourse.tile as tile
from concourse import mybir
from concourse import bass_isa
from concourse.bass_utils import run_bass_kernel_spmd

F32 = mybir.dt.float32
BF16 = mybir.dt.bfloat16
AF = mybir.ActivationFunctionType
ALU = mybir.AluOpType
AX = mybir.AxisListType

# When True, q/k are stored fp8-e4m3 in DoubleRow [K,2,N] layout and the
# score matmuls run one fp8 DoubleRow matmul per (j-tile, i-stripe) at 2x PE
# rate (rel err ~6.7e-3 vs ~9e-4 for bf16; see numpy study).
S_FP8 = True

C = 256          # channels
N = 4096         # spatial positions (64*64)
GROUPS = 32      # groupnorm groups -> 8 channels per group
EPS = 1e-5
SCALE = C ** -0.5
NSTRIPE = 8      # stripes over the spatial dim
SW = N // NSTRIPE  # 512
NJT = N // 128   # 32 j-tiles
GSIZE = (C // GROUPS) * N  # elements per group = 32768


def _emit(nc, tc, d, parts="ABC"):
    """Emit the per-core program. d: dict of DRAM tensor handles."""
    const = tc.alloc_tile_pool(name="const", bufs=1)

    # --- x -> SBUF ---
    # The startup critical path (stats -> scale -> h -> everything) only needs
    # x at bf16 precision: bf16 quantization noise averages out over the
    # 32768-element group stats, and h is consumed in bf16 by the matmuls
    # anyway. So a half-size bf16 copy of x (host-prepared) lands first, and
    # the f32 x needed for the residual ~80us later arrives via the idle
    # gpsimd SWDGE path. bf16 DMAs are emitted before the weights (HWDGE
    # descriptor generation is a shared serial ~0.6us/DMA resource) and
    # spread across both HWDGE issuing engines.
    NSEG = 1
    SEG = N // NSEG
    xbf = []
    x_issuers = [nc.sync, nc.scalar, nc.scalar, nc.sync]
    for t in range(2):
        xb_ = const.tile([128, N], BF16, tag=f"xbf{t}", name=f"xbf{t}")
        for g in range(NSEG):
            x_issuers[t * NSEG + g].dma_start(
                xb_[:, g * SEG:(g + 1) * SEG],
                d["xbf"][t * 128:(t + 1) * 128, g * SEG:(g + 1) * SEG])
        xbf.append(xb_)

    # --- weights / params -> SBUF ---
    # All bf16 weights arrive as one packed [256, 1280] array (2 DMAs):
    # cols 0:768 = qkv_w.T, 768:1024 = proj_w.T, 1024:1280 (rows 0:128) =
    # v-bias broadcast tile. Small f32 params packed into [128, 10] (1 DMA).
    wb = []
    for t in range(2):
        w = const.tile([128, 1280], BF16, tag=f"wb{t}", name=f"wb{t}")
        nc.scalar.dma_start(w[:], d["wbig"][t * 128:(t + 1) * 128, :])
        wb.append(w)
    wq = [wb[0][:, 0:768], wb[1][:, 0:768]]
    wp = [wb[0][:, 768:1024], wb[1][:, 768:1024]]
    vbb = wb[0][:, 1024:1280]
    smalls = const.tile([128, 10], F32, tag="smalls")
    nc.scalar.dma_start(smalls[:], d["smalls"][:])
    qkvb = smalls[:, 0:4]
    projb = smalls[:, 4:6]
    nw = smalls[:, 6:8]
    nb = smalls[:, 8:10]
    gm = const.tile([128, 128], F32, tag="gm")
    nc.scalar.dma_start(gm[:], d["gm"][:])

    # f32 x tiles for the residual — allocated here, but their DMAs are
    # emitted after phase A so the (in-order) DMA resources service the
    # startup-critical bf16 x and weights first.
    xt = [const.tile([128, N], F32, tag=f"x{t}", name=f"x{t}") for t in range(2)]

    # --- phase A: groupnorm stats -> per-channel scale/bias ---
    # Fully per-chunk (chunk-major layout): chunk 0's scale/bias — and with
    # them the first h tiles and qkv matmuls — are ready before chunk 1's
    # stats have even landed.
    # pstats col for (t, kind, g) = (2t+kind)*NSEG + g; stats col = 2t+kind.
    pstats = const.tile([128, 4 * NSEG], F32, tag="pstats")
    stats = const.tile([128, 4], F32, tag="stats")
    scl = const.tile([128, 2], F32, tag="scl")
    bia = const.tile([128, 2], F32, tag="bia")
    gstats_mm = None
    with tc.tile_pool(name="scratch", bufs=2) as scr, \
         tc.tile_pool(name="pa_ps", bufs=1, space="PSUM") as pa_ps:
        for t in range(2):
            for g in range(NSEG):
                seg = xbf[t][:, g * SEG:(g + 1) * SEG]
                c0 = (2 * t + 0) * NSEG + g
                c1 = (2 * t + 1) * NSEG + g
                nc.vector.reduce_sum(pstats[:, c0:c0 + 1], seg, axis=AX.X)
                sq = scr.tile([128, SEG], F32, tag="sq")
                nc.scalar.activation(sq[:], seg, AF.Square,
                                     accum_out=pstats[:, c1:c1 + 1])
            for kind in range(2):
                tk = 2 * t + kind
                nc.vector.reduce_sum(stats[:, tk:tk + 1],
                                     pstats[:, tk * NSEG:(tk + 1) * NSEG], axis=AX.X)
            gstats = pa_ps.tile([128, 2], F32, tag=f"gstats{t}", name=f"gstats{t}")
            gstats_mm = nc.tensor.matmul(gstats[:], gm[:], stats[:, 2 * t:2 * t + 2],
                                         start=True, stop=True)
            # mex cols = [mean, ex2] for this chunk
            mex = const.tile([128, 2], F32, tag=f"mex{t}", name=f"mex{t}")
            nc.vector.tensor_scalar_mul(mex[:], gstats[:], 1.0 / GSIZE)
            mean = mex[:, 0:1]
            ex2 = mex[:, 1:2]
            var = const.tile([128, 1], F32, tag=f"var{t}", name=f"var{t}")
            std = const.tile([128, 1], F32, tag=f"std{t}", name=f"std{t}")
            rstd = const.tile([128, 1], F32, tag=f"rstd{t}", name=f"rstd{t}")
            negm2 = const.tile([128, 1], F32, tag=f"negm2{t}", name=f"negm2{t}")
            nc.vector.scalar_tensor_tensor(negm2[:], mean, -1.0, mean,
                                           op0=ALU.mult, op1=ALU.mult)
            nc.vector.scalar_tensor_tensor(var[:], ex2, EPS, negm2[:],
                                           op0=ALU.add, op1=ALU.add)
            nc.scalar.activation(std[:], var[:], AF.Sqrt)
            nc.vector.reciprocal(rstd[:], std[:])
            nc.vector.tensor_mul(scl[:, t:t + 1], nw[:, t:t + 1], rstd[:])
            mscl = const.tile([128, 1], F32, tag=f"mscl{t}", name=f"mscl{t}")
            nc.vector.tensor_mul(mscl[:], mean, scl[:, t:t + 1])
            nc.vector.tensor_sub(bia[:, t:t + 1], nb[:, t:t + 1], mscl[:])

    # f32 x for the residual — needed from the first stripe tail (~90us in);
    # issued via the idle gpsimd SWDGE path. The explicit dep on the stats
    # matmul keeps its transfers off the (in-order) DMA resources until the
    # startup-critical bf16-x/weights burst is done.
    for t in range(2):
        xdma = nc.gpsimd.dma_start(xt[t][:], d["x"][t * 128:(t + 1) * 128, :])
        _bass_rust.add_dep_helper(xdma.ins, gstats_mm.ins,
                                  reason="delay f32-x past startup DMA burst")

    # --- phase B: h = x*scl+bia (bf16), q,k ([c,n]) and vT ([n,c]) ---
    FP8 = mybir.dt.float8e4
    if S_FP8:
        # q/k in DoubleRow layout: partition p, element e <-> channel e*128+p
        qf8 = const.tile([128, 2, N], FP8, tag="qf8")
        kf8 = const.tile([128, 2, N], FP8, tag="kf8")
        qk = [qf8[:, 0, :], qf8[:, 1, :], kf8[:, 0, :], kf8[:, 1, :]]
    else:
        qk = []
        for i in range(4):  # q0,q1,k0,k1
            t_ = const.tile([128, N], BF16, tag=f"qk{i}", name=f"qk{i}")
            qk.append(t_)
    vt = []
    for j in range(NJT):
        t_ = const.tile([128, 256], BF16, tag=f"vt{j}", name=f"vt{j}")
        vt.append(t_)

    with tc.tile_pool(name="hpool", bufs=1) as hp, \
         tc.tile_pool(name="pb_ps", bufs=3, space="PSUM") as pbp, \
         tc.tile_pool(name="pv_ps", bufs=3, space="PSUM") as pvp:
        # h = x*scl + bia on ACT (per-partition scale/bias APs); pre-emit all
        # 16 tiles so production runs ahead of PE consumption.
        hs = []
        for s in range(NSTRIPE):
            sl = slice(s * SW, (s + 1) * SW)
            hts = []
            for t in range(2):
                ht = hp.tile([128, SW], BF16, tag=f"h{t}_{s}", name=f"h{t}_{s}")
                nc.scalar.activation(ht[:], xbf[t][:, sl], AF.Identity,
                                     bias=bia[:, t:t + 1], scale=scl[:, t:t + 1])
                hts.append(ht)
            hs.append(hts)
        for s in range(NSTRIPE):
            sl = slice(s * SW, (s + 1) * SW)
            hts = hs[s]
            for dt in (2, 3, 0, 1):  # k first: phase C's first matmuls need k
                ps = pbp.tile([128, SW], F32, tag="qkps", name="qkps")
                nc.tensor.matmul(ps[:], wq[0][:, dt * 128:(dt + 1) * 128], hts[0][:],
                                 start=True, stop=False)
                nc.tensor.matmul(ps[:], wq[1][:, dt * 128:(dt + 1) * 128], hts[1][:],
                                 start=False, stop=True)
                # split bias-copies q->DVE, k->ACT so neither engine exceeds
                # PE's ~27us in phase B (ACT also produces the h tiles)
                if dt < 2:
                    nc.vector.tensor_scalar_add(qk[dt][:, sl], ps[:], qkvb[:, dt:dt + 1])
                else:
                    nc.scalar.activation(qk[dt][:, sl], ps[:], AF.Identity,
                                         bias=qkvb[:, dt:dt + 1])
            for n4 in range(4):
                jt = s * 4 + n4
                psv = pvp.tile([128, 256], F32, tag="vtps", name="vtps")
                nc.tensor.matmul(psv[:], hts[0][:, n4 * 128:(n4 + 1) * 128],
                                 wq[0][:, 512:768], start=True, stop=False)
                nc.tensor.matmul(psv[:], hts[1][:, n4 * 128:(n4 + 1) * 128],
                                 wq[1][:, 512:768], start=False, stop=True)
                nc.vector.tensor_add(vt[jt][:], psv[:], vbb[:])

    # --- phase C: attention + proj + residual, per i-stripe ---
    if "C" not in parts:
        # timing variant: still write something to out so nothing is elided
        dummy = const.tile([128, 16], F32, tag="dummy")
        nc.vector.tensor_copy(dummy[:], xt[0][:, 0:16])
        nc.gpsimd.dma_start(d["out"][0:128, 0:16], dummy[:])
        const.release()
        return
    LAG = 6
    with tc.tile_pool(name="wpool", bufs=LAG + 3) as wpo, \
         tc.tile_pool(name="raccp", bufs=3) as rp, \
         tc.tile_pool(name="misc", bufs=2) as mp, \
         tc.tile_pool(name="s_ps", bufs=3, space="PSUM") as sp, \
         tc.tile_pool(name="a_ps", bufs=4, space="PSUM") as apo, \
         tc.tile_pool(name="o_ps", bufs=1, space="PSUM") as opo:

        def make_tail(ist, racc, a_ps):
            """Normalization + proj + residual for a finished stripe, split in
            three parts that are interleaved into the next stripe's matmul
            stream (the serial rsum->recip->mul chain hides behind PE work
            instead of stalling it)."""
            sl = slice(ist * SW, (ist + 1) * SW)
            st = {}

            def part1():
                # all-reduce over partitions on the (idle) gpsimd engine:
                # every partition ends up holding the softmax denominator row
                rall = mp.tile([128, 2 * SW], F32, tag="rall")
                nc.gpsimd.partition_all_reduce(rall[:], racc[:], 128,
                                               bass_isa.ReduceOp.add)
                st["rall"] = rall

            def part2a():
                rall = st["rall"]
                rsum = mp.tile([128, SW], F32, tag="rsum")
                nc.vector.tensor_add(rsum[:], rall[:, 0:SW], rall[:, SW:2 * SW])
                rinv = mp.tile([128, SW], F32, tag="rinv")
                nc.vector.reciprocal(rinv[:], rsum[:])
                st["rinv"] = rinv

            def part2b():
                a_sb = []
                for ct in range(2):
                    t_ = mp.tile([128, SW], BF16, tag=f"asb{ct}", name=f"asb{ct}")
                    nc.vector.tensor_mul(t_[:], a_ps[ct][:], st["rinv"][:])
                    a_sb.append(t_)
                st["a_sb"] = a_sb

            def part2():
                part2a()
                part2b()

            def part3():
                a_sb = st["a_sb"]
                for dt in range(2):
                    o_ps = opo.tile([128, SW], F32, tag="ops", name="ops")
                    nc.tensor.matmul(o_ps[:], wp[0][:, dt * 128:(dt + 1) * 128], a_sb[0][:],
                                     start=True, stop=False)
                    nc.tensor.matmul(o_ps[:], wp[1][:, dt * 128:(dt + 1) * 128], a_sb[1][:],
                                     start=False, stop=True)
                    o_sb = mp.tile([128, SW], F32, tag=f"osb{dt}", name=f"osb{dt}")
                    nc.vector.scalar_tensor_tensor(o_sb[:], o_ps[:], projb[:, dt:dt + 1],
                                                   xt[dt][:, sl], op0=ALU.add, op1=ALU.add)
                    nc.gpsimd.dma_start(d["out"][dt * 128:(dt + 1) * 128, sl], o_sb[:])

            return [part1, part2, part3, part2a, part2b]

        pending = None
        NPAIR = NJT // 2
        PLAG = LAG // 2
        for ist in range(NSTRIPE):
            sl = slice(ist * SW, (ist + 1) * SW)
            racc = rp.tile([128, 2 * SW], F32, tag="racc")
            a_ps = [apo.tile([128, SW], F32, tag="aps", name="aps") for _ in range(2)]
            # exp output halves of two consecutive j-tiles share one SBUF
            # tile, so the racc accumulation runs at [128,1024] granularity
            # (half the DVE per-op overhead) while PSUM stays per-jt
            # single-bank. AV matmuls run LAG steps behind production so the
            # (in-order) PE queue never head-of-line blocks on exp.
            w_pairs = {}
            for jt in range(NJT + LAG):
                if jt < NJT:
                    s_ps = sp.tile([128, SW], F32, tag="sps", name="sps")
                    if S_FP8:
                        nc.tensor.matmul(s_ps[:], kf8[:, :, jt * 128:(jt + 1) * 128],
                                         qf8[:, :, sl], start=True, stop=True,
                                         perf_mode=mybir.MatmulPerfMode.DoubleRow)
                    else:
                        nc.tensor.matmul(s_ps[:], qk[2][:, jt * 128:(jt + 1) * 128],
                                         qk[0][:, sl], start=True, stop=False)
                        nc.tensor.matmul(s_ps[:], qk[3][:, jt * 128:(jt + 1) * 128],
                                         qk[1][:, sl], start=False, stop=True)
                    p = jt // 2
                    if jt % 2 == 0:
                        w_pairs[p] = wpo.tile([128, 2 * SW], BF16, tag="wsb", name="wsb")
                    hsl = slice((jt % 2) * SW, (jt % 2 + 1) * SW)
                    nc.scalar.activation(w_pairs[p][:, hsl], s_ps[:], AF.Exp, scale=SCALE)
                    if jt % 2 == 1:
                        if p == 0:
                            nc.vector.tensor_copy(racc[:], w_pairs[p][:])
                        else:
                            nc.vector.tensor_add(racc[:], racc[:], w_pairs[p][:])
                if pending is not None:
                    if jt == 1:
                        pending[0]()
                    elif jt == 3:
                        pending[1]()
                    elif jt == 7:
                        pending[2]()
                        pending = None
                if ist == NSTRIPE - 1 and "noav" not in parts:
                    # last stripe: run the all-reduce and the fold/recip while
                    # the trailing AV matmuls still execute; only the a_sb
                    # muls and proj remain after the loop.
                    if jt == NJT:
                        last_tail = make_tail(ist, racc, a_ps)
                        last_tail[0]()          # part1: all-reduce
                        pending = None
                    elif jt == NJT + 3:
                        last_tail[3]()          # part2a: fold + reciprocal
                        pending = [last_tail[4], last_tail[2]]  # muls, proj
                if "noav" in parts:
                    continue
                if jt >= LAG:
                    j2 = jt - LAG
                    w2 = w_pairs[j2 // 2]
                    if j2 % 2 == 1:
                        del w_pairs[j2 // 2]
                    hsl = slice((j2 % 2) * SW, (j2 % 2 + 1) * SW)
                    for ct in range(2):
                        nc.tensor.matmul(a_ps[ct][:], vt[j2][:, ct * 128:(ct + 1) * 128],
                                         w2[:, hsl], start=(j2 == 0), stop=(j2 == NJT - 1))
            if "noav" in parts:
                o_sb = mp.tile([128, SW], F32, tag="osb0", name="osb0")
                nc.vector.tensor_add(o_sb[:], racc[:, 0:SW], xt[0][:, sl])
                nc.gpsimd.dma_start(d["out"][0:128, sl], o_sb[:])
                continue
            if ist < NSTRIPE - 1:
                pending = make_tail(ist, racc, a_ps)
        if pending is not None:
            for p in pending:
                p()

    const.release()


def build_program(repeat: int = 1, parts: str = "ABC"):
    nc = bacc.Bacc("TRN2", target_bir_lowering=False, debug=False, num_devices=8)
    d = {
        "x": nc.declare_dram_parameter("x", [C, N], F32, isOutput=False),
        "xbf": nc.declare_dram_parameter("xbf", [C, N], BF16, isOutput=False),
        "wbig": nc.declare_dram_parameter("wbig", [C, 1280], BF16, isOutput=False),
        "smalls": nc.declare_dram_parameter("smalls", [128, 10], F32, isOutput=False),
        "gm": nc.declare_dram_parameter("gm", [128, 128], F32, isOutput=False),
        "out": nc.declare_dram_parameter("out", [C, N], F32, isOutput=True),
    }
    with tile.TileContext(nc) as tc:
        for _ in range(repeat):
            _emit(nc, tc, d, parts)
    nc.compile()
    return nc


def make_in_maps(x, norm_w, norm_b, qkv_w, qkv_b, proj_w, proj_b):
    x = np.asarray(x, np.float32)
    B = x.shape[0]
    qkv_w = np.asarray(qkv_w, np.float32)
    qkv_b = np.asarray(qkv_b, np.float32)
    proj_w = np.asarray(proj_w, np.float32)
    proj_b = np.asarray(proj_b, np.float32)
    wbig = np.zeros((256, 1280), np.float32)
    wbig[:, 0:768] = qkv_w.T
    wbig[:, 768:1024] = proj_w.T
    wbig[0:128, 1024:1280] = np.tile(qkv_b[512:].reshape(1, 256), (128, 1))
    smalls = np.zeros((128, 10), np.float32)
    smalls[:, 0:4] = qkv_b[:512].reshape(4, 128).T
    smalls[:, 4:6] = proj_b.reshape(2, 128).T
    smalls[:, 6:8] = np.asarray(norm_w, np.float32).reshape(2, 128).T
    smalls[:, 8:10] = np.asarray(norm_b, np.float32).reshape(2, 128).T
    shared = {
        "wbig": wbig.astype(ml_dtypes.bfloat16),
        "smalls": smalls,
        "gm": (np.arange(128)[:, None] // 8 == np.arange(128)[None, :] // 8).astype(np.float32),
    }
    return [
        dict(shared,
             x=np.ascontiguousarray(x[b].reshape(C, N)),
             xbf=np.ascontiguousarray(x[b].reshape(C, N)).astype(ml_dtypes.bfloat16))
        for b in range(B)
    ]


_NC_CACHE = {}


def get_program(repeat: int = 1):
    if repeat not in _NC_CACHE:
        _NC_CACHE[repeat] = build_program(repeat)
    return _NC_CACHE[repeat]


def kernel(x, norm_w, norm_b, qkv_w, qkv_b, proj_w, proj_b):
    x = np.asarray(x, np.float32)
    B, C_, H_, W_ = x.shape
    in_maps = make_in_maps(x, norm_w, norm_b, qkv_w, qkv_b, proj_w, proj_b)
    nc = get_program()
    res = run_bass_kernel_spmd(nc, in_maps, core_ids=list(range(len(in_maps))))
    out = np.stack([np.asarray(res.results[b]["out"], np.float32) for b in range(B)])
    return out.reshape(B, C_, H_, W_)



# revision 19
# speedup vs baseline: 1.4528x; 1.1024x over previous
"""Trainium2 Bass kernel for nn_AttentionBlock (GroupNorm + spatial self-attention
+ residual). Full inputs in, full outputs out; data-parallel over batch (B=8)
across 8 NeuronCores; each core processes one [C=256, N=4096] image.

Design (full-fp8 attention; see study.py for the precision budget):
  - All big matmuls run fp8-e4m3 DoubleRow (0.5 cycles/row, 4x less PE time
    than two bf16 chunk matmuls for a 256-contraction). Weights are scaled
    x16 on the host so their sigma~1 lands in fp8's normal range; the score
    scale absorbs 1/256 and the softmax denominator absorbs the v-side 16.
  - DR layouts are [128, 2, n] with channel = r*128 + p.
  - q = 16(q_mm + bq) fp8; k = 16 k_mm fp8 (k-bias provably cancels in the
    softmax over j); v = 16 v_mm fp8 transposed to [n, c] pairs (v-bias passes
    through softmax and is folded into proj_b on the host).
  - scores s' = k'^T q' = 256 s, PSUM f32; w = exp(s'*SCALE/256 - 4) in fp8:
    the e^-4 offset keeps fp8 in range and cancels in normalization. exp runs
    split across two engines: ACT (exact exp, fp8 out) for the first
    N_ACT_PAIRS j-pairs of each stripe, DVE for the rest via a Schraudolph
    bit-trick (uint8 = rne(8/ln2*u + 56) is the fp8-e4m3 encoding of e^u;
    DVE's f32->uint8 convert rounds+saturates, verified on HW).
  - softmax denominator: DR ones-matmul (value 16 = v descale) accumulating
    [1, SW] in PSUM; reciprocal_approx_fast on DVE; partition_broadcast on
    the (otherwise idle) Pool engine; a = a_ps * rinv -> fp8 pairs (DVE).
  - proj fp8 DR; o = Identity(o_ps/16 + projb') on ACT; residual add on Pool;
    SWDGE DMA out.
"""

import sys

try:
    import concourse  # noqa: F401
except ImportError:
    sys.path.insert(0, "/opt/trn_rl_repo")

import numpy as np
import ml_dtypes

import bass_rust as _bass_rust
import concourse.bacc as bacc
import concourse.tile as tile
from concourse import mybir
from concourse import bass_isa
from concourse.bass_utils import run_bass_kernel_spmd

F32 = mybir.dt.float32
BF16 = mybir.dt.bfloat16
FP8 = mybir.dt.float8e4
U8 = mybir.dt.uint8
AF = mybir.ActivationFunctionType
ALU = mybir.AluOpType
AX = mybir.AxisListType
DR = mybir.MatmulPerfMode.DoubleRow

C = 256          # channels
N = 4096         # spatial positions
GROUPS = 32
EPS = 1e-5
SCALE = C ** -0.5
SC2 = SCALE / 256.0          # fold the x16 q/k weight scaling out of scores
OFFSET = -4.0                # exp offset; cancels in softmax normalization
SCHRA_A = 8.0 / np.log(2.0)  # fp8-e4m3 schraudolph slope
NSTRIPE = 8
SW = N // NSTRIPE            # 512
NPAIR = N // 256             # 16 j-pairs (j-tiles of 128, two per pair)
PLAG = 5                     # pairs of lag between exp production and AV/ones
NJT = N // 128               # 32 j-tiles per stripe
# exp engine per j-tile, spread so ACT (faster, ~18 tiles) and DVE (~14) run
# concurrently within a stripe (a contiguous split serializes them through
# the score-buffer WAR).
ACT_JTS = frozenset(j for j in range(NJT) if (j * 18) // NJT != ((j + 1) * 18) // NJT)
GSIZE = (C // GROUPS) * N


def _emit(nc, tc, d):
    const = tc.alloc_tile_pool(name="const", bufs=1)

    # --- input DMAs: bf16 x first (startup critical), weights, then f32 x ---
    xbf = []
    x_issuers = [nc.sync, nc.scalar]
    for t in range(2):
        xb_ = const.tile([128, N], BF16, tag=f"xbf{t}", name=f"xbf{t}")
        x_issuers[t].dma_start(xb_[:], d["xbf"][t * 128:(t + 1) * 128, :])
        xbf.append(xb_)

    w8 = const.tile([128, 2, 1024], FP8, tag="w8")
    nc.scalar.dma_start(w8[:].bitcast(U8), d["w8"][:])
    wq8 = w8[:, :, 0:768]     # q 0:256, k 256:512, v 512:768 (x16 scaled)
    wp8 = w8[:, :, 768:1024]  # proj (x16 scaled)
    smalls = const.tile([128, 8], F32, tag="smalls")
    nc.scalar.dma_start(smalls[:], d["smalls"][:])
    qb16 = smalls[:, 0:2]
    projb = smalls[:, 2:4]
    nw = smalls[:, 4:6]
    nb = smalls[:, 6:8]
    gm = const.tile([128, 128], F32, tag="gm")
    nc.scalar.dma_start(gm[:], d["gm"][:])

    ones16 = const.tile([128, 2, 16], FP8, tag="ones16")
    nc.vector.memset(ones16[:], 16.0)
    negoff = const.tile([128, 1], F32, tag="negoff")
    nc.vector.memset(negoff[:], OFFSET)

    # f32 x for the residual; DMAs delayed past the startup burst (below).
    xt = [const.tile([128, N], F32, tag=f"x{t}", name=f"x{t}") for t in range(2)]

    # --- phase A: groupnorm stats -> per-chunk scale/bias ---
    stats = const.tile([128, 4], F32, tag="stats")
    scl = const.tile([128, 2], F32, tag="scl")
    bia = const.tile([128, 2], F32, tag="bia")
    gstats_mm = None
    with tc.tile_pool(name="scratch", bufs=2) as scr, \
         tc.tile_pool(name="pa_ps", bufs=1, space="PSUM") as pa_ps:
        for t in range(2):
            nc.vector.reduce_sum(stats[:, 2 * t:2 * t + 1], xbf[t][:], axis=AX.X)
            sq = scr.tile([128, N], F32, tag="sq")
            nc.scalar.activation(sq[:], xbf[t][:], AF.Square,
                                 accum_out=stats[:, 2 * t + 1:2 * t + 2])
            gstats = pa_ps.tile([128, 2], F32, tag=f"gstats{t}", name=f"gstats{t}")
            gstats_mm = nc.tensor.matmul(gstats[:], gm[:], stats[:, 2 * t:2 * t + 2],
                                         start=True, stop=True)
            mex = const.tile([128, 2], F32, tag=f"mex{t}", name=f"mex{t}")
            nc.vector.tensor_scalar_mul(mex[:], gstats[:], 1.0 / GSIZE)
            mean = mex[:, 0:1]
            ex2 = mex[:, 1:2]
            var = const.tile([128, 1], F32, tag=f"var{t}", name=f"var{t}")
            std = const.tile([128, 1], F32, tag=f"std{t}", name=f"std{t}")
            rstd = const.tile([128, 1], F32, tag=f"rstd{t}", name=f"rstd{t}")
            negm2 = const.tile([128, 1], F32, tag=f"negm2{t}", name=f"negm2{t}")
            nc.vector.scalar_tensor_tensor(negm2[:], mean, -1.0, mean,
                                           op0=ALU.mult, op1=ALU.mult)
            nc.vector.scalar_tensor_tensor(var[:], ex2, EPS, negm2[:],
                                           op0=ALU.add, op1=ALU.add)
            nc.scalar.activation(std[:], var[:], AF.Sqrt)
            nc.vector.reciprocal(rstd[:], std[:])
            nc.vector.tensor_mul(scl[:, t:t + 1], nw[:, t:t + 1], rstd[:])
            mscl = const.tile([128, 1], F32, tag=f"mscl{t}", name=f"mscl{t}")
            nc.vector.tensor_mul(mscl[:], mean, scl[:, t:t + 1])
            nc.vector.tensor_sub(bia[:, t:t + 1], nb[:, t:t + 1], mscl[:])
        # preload the exp table set while phase B runs (one-time ~2.7us)
        warm = const.tile([128, 1], F32, tag="expwarm")
        nc.scalar.activation(warm[:], stats[:, 0:1], AF.Exp, scale=0.0)

    for t in range(2):
        xdma = nc.gpsimd.dma_start(xt[t][:], d["x"][t * 128:(t + 1) * 128, :])
        _bass_rust.add_dep_helper(xdma.ins, gstats_mm.ins,
                                  reason="delay f32-x past startup DMA burst")

    # --- phase B: h (fp8 DR), q/k fp8 DR per-stripe tiles, vT fp8 pairs ---
    # Per-stripe q/k tiles keep the dependency granularity fine enough that
    # phase C's first score matmuls start as soon as phase B's first stripe
    # lands, instead of waiting for all of q/k.
    qt = [const.tile([128, 2, SW], FP8, tag=f"qt{s}", name=f"qt{s}")
          for s in range(NSTRIPE)]
    kt = [const.tile([128, 2, SW], FP8, tag=f"kt{s}", name=f"kt{s}")
          for s in range(NSTRIPE)]
    vtp = []
    for jp in range(NPAIR):
        vtp.append(const.tile([128, 2, 256], FP8, tag=f"vtp{jp}", name=f"vtp{jp}"))

    with tc.tile_pool(name="hpool", bufs=3) as hp, \
         tc.tile_pool(name="pbk_ps", bufs=2, space="PSUM") as pbk, \
         tc.tile_pool(name="pbq_ps", bufs=2, space="PSUM") as pbq, \
         tc.tile_pool(name="pbv_ps", bufs=2, space="PSUM") as pbv:
        for s in range(NSTRIPE):
            sl = slice(s * SW, (s + 1) * SW)
            ht = hp.tile([128, 2, SW], FP8, tag="h", name="h")
            for t in range(2):
                nc.scalar.activation(ht[:, t, :], xbf[t][:, sl], AF.Identity,
                                     bias=bia[:, t:t + 1], scale=scl[:, t:t + 1])
            # k (no bias: it cancels in the softmax over j)
            kps = pbk.tile([128, 1024], F32, tag="kps", name="kps")
            for t in range(2):
                nc.tensor.matmul(kps[:, t * SW:(t + 1) * SW],
                                 wq8[:, :, 256 + t * 128:256 + (t + 1) * 128],
                                 ht[:], start=True, stop=True, perf_mode=DR)
            nc.scalar.activation(kt[s][:, :, :], kps[:], AF.Copy)
            # q (+16*bias)
            for t in range(2):
                qps = pbq.tile([128, SW], F32, tag="qps", name="qps")
                nc.tensor.matmul(qps[:], wq8[:, :, t * 128:(t + 1) * 128],
                                 ht[:], start=True, stop=True, perf_mode=DR)
                nc.vector.tensor_scalar_add(qt[s][:, t, :], qps[:], qb16[:, t:t + 1])
            # vT pairs: two n4-chunks of 128 -> one [128, 512] psum -> one copy
            for half in range(2):
                vps = pbv.tile([128, 512], F32, tag="vps", name="vps")
                for par in range(2):
                    n4 = half * 2 + par
                    nc.tensor.matmul(vps[:, par * 256:(par + 1) * 256],
                                     ht[:, :, n4 * 128:(n4 + 1) * 128],
                                     wq8[:, :, 512:768], start=True, stop=True,
                                     perf_mode=DR)
                jp = s * 2 + half
                nc.vector.tensor_copy(vtp[jp][:, :, :], vps[:])

    # --- phase C: attention + proj + residual, per i-stripe ---
    # PSUM: s_ps 5 banks + a_ps 2 + do_ps 1 (denominator and o_ps share one
    # bank; their lifetimes are disjoint within the stripe-tail rotation).
    with tc.tile_pool(name="wpool", bufs=PLAG + 3) as wpo, \
         tc.tile_pool(name="misc", bufs=3) as mp, \
         tc.tile_pool(name="s_ps", bufs=5, space="PSUM") as spo, \
         tc.tile_pool(name="a_ps", bufs=2, space="PSUM") as apo, \
         tc.tile_pool(name="do_ps", bufs=1, space="PSUM") as dpo:
        opo = dpo

        def make_tail(ist, denom, a_ps):
            sl = slice(ist * SW, (ist + 1) * SW)
            st = {}

            def part1():
                rinv = mp.tile([1, SW], F32, tag="rinv")
                nc.vector.reciprocal_approx_fast(out=rinv[:], in_=denom[:])
                rb = mp.tile([128, SW], F32, tag="rb")
                nc.gpsimd.partition_broadcast(rb[:], rinv[:], channels=128)
                st["rb"] = rb

            def part2():
                a8 = mp.tile([128, 2, SW], FP8, tag="a8")
                for ct in range(2):
                    nc.vector.tensor_mul(a8[:, ct, :], a_ps[ct][:], st["rb"][:])
                st["a8"] = a8

            def part3():
                a8 = st["a8"]
                for dt in range(2):
                    o_ps = opo.tile([128, SW], F32, tag="do", name="ops")
                    nc.tensor.matmul(o_ps[:], wp8[:, :, dt * 128:(dt + 1) * 128],
                                     a8[:], start=True, stop=True, perf_mode=DR)
                    o_t = mp.tile([128, SW], F32, tag=f"ot{dt}", name=f"ot{dt}")
                    nc.scalar.activation(o_t[:], o_ps[:], AF.Identity,
                                         bias=projb[:, dt:dt + 1], scale=1.0 / 16.0)
                    o_sb = mp.tile([128, SW], F32, tag=f"osb{dt}", name=f"osb{dt}")
                    nc.gpsimd.tensor_tensor(o_sb[:], o_t[:], xt[dt][:, sl],
                                            op=ALU.add)
                    # out stores on the SP HWDGE queue: SWDGE desc-gen on the
                    # Pool sequencer (~8us each) would block the broadcast and
                    # residual ops queued behind it.
                    nc.sync.dma_start(d["out"][dt * 128:(dt + 1) * 128, sl], o_sb[:])

            return [part1, part2, part3]

        pending = None
        for ist in range(NSTRIPE):
            isl = slice(ist * SW, (ist + 1) * SW)
            denom = None
            a_ps = None
            wpairs = {}
            for p in range(NPAIR + PLAG):
                if p < NPAIR:
                    wt = wpo.tile([128, 2, SW], FP8, tag="wp", name="wp")
                    for r in range(2):
                        jt = 2 * p + r
                        s_t = spo.tile([128, SW], F32, tag="sps", name="sps")
                        nc.tensor.matmul(s_t[:],
                                         kt[jt // 4][:, :, (jt % 4) * 128:(jt % 4 + 1) * 128],
                                         qt[ist][:], start=True, stop=True,
                                         perf_mode=DR)
                        if jt in ACT_JTS:
                            nc.scalar.activation(wt[:, r, :], s_t[:], AF.Exp,
                                                 bias=negoff[:], scale=SC2)
                        else:
                            nc.vector.tensor_scalar(
                                wt[:, r, :].bitcast(U8), s_t[:],
                                SCHRA_A * SC2, 56.0 + OFFSET * SCHRA_A,
                                op0=ALU.mult, op1=ALU.add)
                    wpairs[p] = wt
                if pending is not None:
                    if p == 0:
                        pending[0]()
                    elif p == 1:
                        pending[1]()
                    elif p == 2:
                        pending[2]()
                        pending = None
                if p == 3:
                    # allocated here (not at stripe start) so the shared
                    # den/o_ps bank's rotation order matches temporal order
                    denom = dpo.tile([1, SW], F32, tag="do", name="den")
                if p >= PLAG:
                    p2 = p - PLAG
                    w2 = wpairs.pop(p2)
                    if a_ps is None:
                        a_ps = [apo.tile([128, SW], F32, tag="aps", name="aps")
                                for _ in range(2)]
                    nc.tensor.matmul(denom[:], ones16[:, :, 0:1], w2[:],
                                     start=(p2 == 0), stop=(p2 == NPAIR - 1),
                                     perf_mode=DR)
                    for ct in range(2):
                        nc.tensor.matmul(a_ps[ct][:],
                                         vtp[p2][:, :, ct * 128:(ct + 1) * 128],
                                         w2[:], start=(p2 == 0),
                                         stop=(p2 == NPAIR - 1), perf_mode=DR)
            pending = make_tail(ist, denom, a_ps)
        for part in pending:
            part()

    const.release()


def build_program(repeat: int = 1):
    nc = bacc.Bacc("TRN2", target_bir_lowering=False, debug=False, num_devices=8)
    d = {
        "x": nc.declare_dram_parameter("x", [C, N], F32, isOutput=False),
        "xbf": nc.declare_dram_parameter("xbf", [C, N], BF16, isOutput=False),
        "w8": nc.declare_dram_parameter("w8", [128, 2048], U8, isOutput=False),
        "smalls": nc.declare_dram_parameter("smalls", [128, 8], F32, isOutput=False),
        "gm": nc.declare_dram_parameter("gm", [128, 128], F32, isOutput=False),
        "out": nc.declare_dram_parameter("out", [C, N], F32, isOutput=True),
    }
    with tile.TileContext(nc) as tc:
        for _ in range(repeat):
            _emit(nc, tc, d)
    nc.compile()
    return nc


def make_in_maps(x, norm_w, norm_b, qkv_w, qkv_b, proj_w, proj_b):
    x = np.asarray(x, np.float32)
    B = x.shape[0]
    qkv_w = np.asarray(qkv_w, np.float32)
    qkv_b = np.asarray(qkv_b, np.float32)
    proj_w = np.asarray(proj_w, np.float32)
    proj_b = np.asarray(proj_b, np.float32)
    FP8NP = ml_dtypes.float8_e4m3

    qkvT = (16.0 * qkv_w).T                      # [256, 768]
    projT = (16.0 * proj_w).T                    # [256, 256]
    wall = np.concatenate([qkvT, projT], axis=1)  # [256, 1024]
    w8 = wall.reshape(2, 128, 1024).transpose(1, 0, 2)  # [p, r, d]
    w8 = np.ascontiguousarray(w8.astype(FP8NP).view(np.uint8).reshape(128, 2048))

    projb_f = proj_b + proj_w @ qkv_b[2 * C:]    # fold v-bias into proj bias
    smalls = np.zeros((128, 8), np.float32)
    smalls[:, 0:2] = (16.0 * qkv_b[:C]).reshape(2, 128).T
    smalls[:, 2:4] = projb_f.reshape(2, 128).T
    smalls[:, 4:6] = np.asarray(norm_w, np.float32).reshape(2, 128).T
    smalls[:, 6:8] = np.asarray(norm_b, np.float32).reshape(2, 128).T
    shared = {
        "w8": w8,
        "smalls": smalls,
        "gm": (np.arange(128)[:, None] // 8 == np.arange(128)[None, :] // 8).astype(np.float32),
    }
    return [
        dict(shared,
             x=np.ascontiguousarray(x[b].reshape(C, N)),
             xbf=np.ascontiguousarray(x[b].reshape(C, N)).astype(ml_dtypes.bfloat16))
        for b in range(B)
    ]


_NC_CACHE = {}


def get_program(repeat: int = 1):
    if repeat not in _NC_CACHE:
        _NC_CACHE[repeat] = build_program(repeat)
    return _NC_CACHE[repeat]


def kernel(x, norm_w, norm_b, qkv_w, qkv_b, proj_w, proj_b):
    x = np.asarray(x, np.float32)
    B, C_, H_, W_ = x.shape
    in_maps = make_in_maps(x, norm_w, norm_b, qkv_w, qkv_b, proj_w, proj_b)
    nc = get_program()
    res = run_bass_kernel_spmd(nc, in_maps, core_ids=list(range(len(in_maps))))
    out = np.stack([np.asarray(res.results[b]["out"], np.float32) for b in range(B)])
    return out.reshape(B, C_, H_, W_)


# revision 31
# speedup vs baseline: 1.8940x; 1.3037x over previous
"""Trainium2 Bass kernel for nn_AttentionBlock (GroupNorm + spatial self-attention
+ residual). Full inputs in, full outputs out; data-parallel over batch (B=8)
across 8 NeuronCores; each core processes one [C=256, N=4096] image.

Design (full-fp8 attention; see study.py for the precision budget):
  - All big matmuls run fp8-e4m3 DoubleRow (0.5 cycles/row, 4x less PE time
    than two bf16 chunk matmuls for a 256-contraction). Weights are scaled
    x16 on the host so their sigma~1 lands in fp8's normal range; the score
    scale absorbs 1/256 and the softmax denominator absorbs the v-side 16.
  - DR layouts are [128, 2, n] with channel = r*128 + p.
  - q = 16(q_mm + bq) fp8; k = 16 k_mm fp8 (k-bias provably cancels in the
    softmax over j); v = 16 v_mm fp8 transposed to [n, c] pairs (v-bias passes
    through softmax and is folded into proj_b on the host).
  - scores s' = k'^T q' = 256 s, PSUM f32; w = exp(s'*SCALE/256 - 4) in fp8:
    the e^-4 offset keeps fp8 in range and cancels in normalization. exp runs
    split across two engines: ACT (exact exp, fp8 out) for the first
    N_ACT_PAIRS j-pairs of each stripe, DVE for the rest via a Schraudolph
    bit-trick (uint8 = rne(8/ln2*u + 56) is the fp8-e4m3 encoding of e^u;
    DVE's f32->uint8 convert rounds+saturates, verified on HW).
  - softmax denominator: DR ones-matmul (value 16 = v descale) accumulating
    [1, SW] in PSUM; reciprocal_approx_fast on DVE; partition_broadcast on
    the (otherwise idle) Pool engine; a = a_ps * rinv -> fp8 pairs (DVE).
  - proj fp8 DR; o = Identity(o_ps/16 + projb') on ACT; residual add on Pool;
    SWDGE DMA out.
"""

import sys

try:
    import concourse  # noqa: F401
except ImportError:
    sys.path.insert(0, "/opt/trn_rl_repo")

import numpy as np
import ml_dtypes

import bass_rust as _bass_rust
import concourse.bacc as bacc
import concourse.tile as tile
from concourse import mybir
from concourse import bass_isa
from concourse.bass_utils import run_bass_kernel_spmd

F32 = mybir.dt.float32
BF16 = mybir.dt.bfloat16
FP8 = mybir.dt.float8e4
U8 = mybir.dt.uint8
AF = mybir.ActivationFunctionType
ALU = mybir.AluOpType
AX = mybir.AxisListType
DR = mybir.MatmulPerfMode.DoubleRow

C = 256          # channels
N = 4096         # spatial positions
GROUPS = 32
EPS = 1e-5
SCALE = C ** -0.5
SC2 = SCALE / 256.0          # fold the x16 q/k weight scaling out of scores
OFFSET = -4.0                # exp offset; cancels in softmax normalization
SCHRA_A = 8.0 / np.log(2.0)  # fp8-e4m3 schraudolph slope
NSTRIPE = 8
SW = N // NSTRIPE            # 512
NPAIR = N // 256             # 16 j-pairs (j-tiles of 128, two per pair)
PLAG = 6                     # pairs of lag between exp production and AV/ones
NJT = N // 128               # 32 j-tiles per stripe
# exp engine per j-tile, spread so ACT (faster, ~18 tiles) and DVE (~14) run
# concurrently within a stripe (a contiguous split serializes them through
# the score-buffer WAR).
ACT_JTS = frozenset(j for j in range(NJT) if (j * 18) // NJT != ((j + 1) * 18) // NJT)
GSIZE = (C // GROUPS) * N


def _emit(nc, tc, d):
    const = tc.alloc_tile_pool(name="const", bufs=1)

    # --- input DMAs: bf16 x split across both HWDGE queues, then weights ---
    xbf = []
    for t in range(2):
        xb_ = const.tile([128, N], BF16, tag=f"xbf{t}", name=f"xbf{t}")
        for g, eng in enumerate((nc.sync, nc.scalar)):
            eng.dma_start(xb_[:, g * (N // 2):(g + 1) * (N // 2)],
                          d["xbf"][t * 128:(t + 1) * 128,
                                   g * (N // 2):(g + 1) * (N // 2)])
        xbf.append(xb_)

    w8 = const.tile([128, 2, 1024], FP8, tag="w8")
    nc.scalar.dma_start(w8[:].bitcast(U8), d["w8"][:])
    wq8 = w8[:, :, 0:768]     # q 0:256, k 256:512, v 512:768 (x16 scaled)
    wp8 = w8[:, :, 768:1024]  # proj (x16 scaled)
    smalls = const.tile([128, 8], F32, tag="smalls")
    nc.scalar.dma_start(smalls[:], d["smalls"][:])
    qb16 = smalls[:, 0:2]
    projb = smalls[:, 2:4]
    nw = smalls[:, 4:6]
    nb = smalls[:, 6:8]
    gm = const.tile([128, 128], F32, tag="gm")
    nc.scalar.dma_start(gm[:], d["gm"][:])

    i16 = const.tile([128, 128], BF16, tag="i16")
    nc.scalar.dma_start(i16[:], d["i16"][:])
    ones16 = const.tile([128, 2, 16], FP8, tag="ones16")
    nc.vector.memset(ones16[:], 16.0)
    negoff = const.tile([128, 1], F32, tag="negoff")
    nc.vector.memset(negoff[:], OFFSET)
    onecol = const.tile([1, 128], F32, tag="onecol")
    nc.vector.memset(onecol[:], 1.0)

    # --- phase A: groupnorm stats -> per-chunk scale/bias ---
    stats = const.tile([128, 4], F32, tag="stats")
    scl = const.tile([128, 2], F32, tag="scl")
    bia = const.tile([128, 2], F32, tag="bia")
    gstats_mm = None
    with tc.tile_pool(name="scratch", bufs=2) as scr, \
         tc.tile_pool(name="pa_ps", bufs=1, space="PSUM") as pa_ps:
        for t in range(2):
            nc.vector.reduce_sum(stats[:, 2 * t:2 * t + 1], xbf[t][:], axis=AX.X)
            sq = scr.tile([128, N], F32, tag="sq")
            nc.scalar.activation(sq[:], xbf[t][:], AF.Square,
                                 accum_out=stats[:, 2 * t + 1:2 * t + 2])
            gstats = pa_ps.tile([128, 2], F32, tag=f"gstats{t}", name=f"gstats{t}")
            gstats_mm = nc.tensor.matmul(gstats[:], gm[:], stats[:, 2 * t:2 * t + 2],
                                         start=True, stop=True)
            mex = const.tile([128, 2], F32, tag=f"mex{t}", name=f"mex{t}")
            nc.vector.tensor_scalar_mul(mex[:], gstats[:], 1.0 / GSIZE)
            mean = mex[:, 0:1]
            ex2 = mex[:, 1:2]
            var = const.tile([128, 1], F32, tag=f"var{t}", name=f"var{t}")
            std = const.tile([128, 1], F32, tag=f"std{t}", name=f"std{t}")
            rstd = const.tile([128, 1], F32, tag=f"rstd{t}", name=f"rstd{t}")
            negm2 = const.tile([128, 1], F32, tag=f"negm2{t}", name=f"negm2{t}")
            nc.vector.scalar_tensor_tensor(negm2[:], mean, -1.0, mean,
                                           op0=ALU.mult, op1=ALU.mult)
            nc.vector.scalar_tensor_tensor(var[:], ex2, EPS, negm2[:],
                                           op0=ALU.add, op1=ALU.add)
            nc.scalar.activation(std[:], var[:], AF.Sqrt)
            nc.vector.reciprocal(rstd[:], std[:])
            nc.vector.tensor_mul(scl[:, t:t + 1], nw[:, t:t + 1], rstd[:])
            mscl = const.tile([128, 1], F32, tag=f"mscl{t}", name=f"mscl{t}")
            nc.vector.tensor_mul(mscl[:], mean, scl[:, t:t + 1])
            nc.vector.tensor_sub(bia[:, t:t + 1], nb[:, t:t + 1], mscl[:])
        # preload the exp table set while phase B runs (one-time ~2.7us)
        warm = const.tile([128, 1], F32, tag="expwarm")
        nc.scalar.activation(warm[:], stats[:, 0:1], AF.Exp, scale=0.0)

    # --- phase B: h (fp8 DR), q/k fp8 DR per-stripe tiles, vT fp8 pairs ---
    # Per-stripe q/k tiles keep the dependency granularity fine enough that
    # phase C's first score matmuls start as soon as phase B's first stripe
    # lands, instead of waiting for all of q/k.
    qt = [const.tile([128, 2, SW], FP8, tag=f"qt{s}", name=f"qt{s}")
          for s in range(NSTRIPE)]
    kt = [const.tile([128, 2, SW], FP8, tag=f"kt{s}", name=f"kt{s}")
          for s in range(NSTRIPE)]
    vtp = []
    for jp in range(NPAIR):
        vtp.append(const.tile([128, 2, 256], FP8, tag=f"vtp{jp}", name=f"vtp{jp}"))

    with tc.tile_pool(name="hpool", bufs=3) as hp, \
         tc.tile_pool(name="pbk_ps", bufs=2, space="PSUM") as pbk, \
         tc.tile_pool(name="pbq_ps", bufs=2, space="PSUM") as pbq, \
         tc.tile_pool(name="pbv_ps", bufs=2, space="PSUM") as pbv:
        for s in range(NSTRIPE):
            sl = slice(s * SW, (s + 1) * SW)
            ht = hp.tile([128, 2, SW], FP8, tag="h", name="h")
            for t in range(2):
                nc.scalar.activation(ht[:, t, :], xbf[t][:, sl], AF.Identity,
                                     bias=bia[:, t:t + 1], scale=scl[:, t:t + 1])
            # k (no bias: it cancels in the softmax over j)
            kps = pbk.tile([128, 1024], F32, tag="kps", name="kps")
            for t in range(2):
                nc.tensor.matmul(kps[:, t * SW:(t + 1) * SW],
                                 wq8[:, :, 256 + t * 128:256 + (t + 1) * 128],
                                 ht[:], start=True, stop=True, perf_mode=DR)
            nc.scalar.activation(kt[s][:, :, :], kps[:], AF.Copy)
            # q (+16*bias)
            for t in range(2):
                qps = pbq.tile([128, SW], F32, tag="qps", name="qps")
                nc.tensor.matmul(qps[:], wq8[:, :, t * 128:(t + 1) * 128],
                                 ht[:], start=True, stop=True, perf_mode=DR)
                nc.vector.tensor_scalar_add(qt[s][:, t, :], qps[:], qb16[:, t:t + 1])
            # vT pairs: two n4-chunks of 128 -> one [128, 512] psum -> one copy
            for half in range(2):
                vps = pbv.tile([128, 512], F32, tag="vps", name="vps")
                for par in range(2):
                    n4 = half * 2 + par
                    nc.tensor.matmul(vps[:, par * 256:(par + 1) * 256],
                                     ht[:, :, n4 * 128:(n4 + 1) * 128],
                                     wq8[:, :, 512:768], start=True, stop=True,
                                     perf_mode=DR)
                jp = s * 2 + half
                nc.vector.tensor_copy(vtp[jp][:, :, :], vps[:])

    # --- phase C: attention + proj + residual, per i-stripe ---
    # PSUM: s_ps 5 banks + a_ps 2 + do_ps 1 (denominator and o_ps share one
    # bank; their lifetimes are disjoint within the stripe-tail rotation).
    with tc.tile_pool(name="wpool", bufs=PLAG + 3) as wpo, \
         tc.tile_pool(name="misc", bufs=3) as mp, \
         tc.tile_pool(name="s_ps", bufs=5, space="PSUM") as spo, \
         tc.tile_pool(name="a_ps", bufs=2, space="PSUM") as apo, \
         tc.tile_pool(name="do_ps", bufs=1, space="PSUM") as dpo:
        opo = dpo

        def make_tail(ist, denom, a_ps):
            sl = slice(ist * SW, (ist + 1) * SW)
            st = {}

            def part1():
                rinv = mp.tile([1, SW], F32, tag="rinv")
                nc.vector.reciprocal_approx_fast(out=rinv[:], in_=denom[:])
                # broadcast rinv to all partitions with a rank-1 f32 matmul
                # into the shared do-bank (its lifetime fits between the
                # denominator read and the o_ps writes)
                rb = mp.tile([128, SW], F32, tag="rb")
                nc.gpsimd.partition_broadcast(rb[:], rinv[:], channels=128)
                st["rb"] = rb

            def part2():
                a8 = mp.tile([128, 2, SW], FP8, tag="a8")
                for ct in range(2):
                    nc.vector.tensor_mul(a8[:, ct, :], a_ps[ct][:], st["rb"][:])
                st["a8"] = a8

            def part3():
                a8 = st["a8"]
                for dt in range(2):
                    o_ps = opo.tile([128, SW], F32, tag="do", name="ops")
                    nc.tensor.matmul(o_ps[:], wp8[:, :, dt * 128:(dt + 1) * 128],
                                     a8[:], start=True, stop=False, perf_mode=DR)
                    # residual: o_ps += 16*xbf via identity matmul (x16 so the
                    # 1/16 proj descale below leaves x unscaled)
                    nc.tensor.matmul(o_ps[:], i16[:], xbf[dt][:, sl],
                                     start=False, stop=True)
                    o_t = mp.tile([128, SW], F32, tag=f"ot{dt}", name=f"ot{dt}")
                    nc.scalar.activation(o_t[:], o_ps[:], AF.Identity,
                                         bias=projb[:, dt:dt + 1], scale=1.0 / 16.0)
                    nc.sync.dma_start(d["out"][dt * 128:(dt + 1) * 128, sl], o_t[:])

            return [part1, part2, part3]

        pending = None
        for ist in range(NSTRIPE):
            isl = slice(ist * SW, (ist + 1) * SW)
            denom = None
            a_ps = None
            wpairs = {}
            for p in range(NPAIR + PLAG):
                if p < NPAIR:
                    wt = wpo.tile([128, 2, SW], FP8, tag="wp", name="wp")
                    for r in range(2):
                        jt = 2 * p + r
                        s_t = spo.tile([128, SW], F32, tag="sps", name="sps")
                        nc.tensor.matmul(s_t[:],
                                         kt[jt // 4][:, :, (jt % 4) * 128:(jt % 4 + 1) * 128],
                                         qt[ist][:], start=True, stop=True,
                                         perf_mode=DR)
                        if jt in ACT_JTS:
                            nc.scalar.activation(wt[:, r, :], s_t[:], AF.Exp,
                                                 bias=negoff[:], scale=SC2)
                        else:
                            nc.vector.tensor_scalar(
                                wt[:, r, :].bitcast(U8), s_t[:],
                                SCHRA_A * SC2, 56.0 + OFFSET * SCHRA_A,
                                op0=ALU.mult, op1=ALU.add)
                    wpairs[p] = wt
                if pending is not None:
                    if p == 0:
                        pending[0]()
                    elif p == 1:
                        pending[1]()
                    elif p == 2:
                        pending[2]()
                        pending = None
                if p == 3:
                    # allocated here (not at stripe start) so the shared
                    # den/o_ps bank's rotation order matches temporal order
                    denom = dpo.tile([1, SW], F32, tag="do", name="den")
                if p >= PLAG:
                    p2 = p - PLAG
                    w2 = wpairs.pop(p2)
                    if a_ps is None:
                        a_ps = [apo.tile([128, SW], F32, tag="aps", name="aps")
                                for _ in range(2)]
                    nc.tensor.matmul(denom[:], ones16[:, :, 0:1], w2[:],
                                     start=(p2 == 0), stop=(p2 == NPAIR - 1),
                                     perf_mode=DR)
                    for ct in range(2):
                        nc.tensor.matmul(a_ps[ct][:],
                                         vtp[p2][:, :, ct * 128:(ct + 1) * 128],
                                         w2[:], start=(p2 == 0),
                                         stop=(p2 == NPAIR - 1), perf_mode=DR)
            pending = make_tail(ist, denom, a_ps)
        for part in pending:
            part()

    const.release()


def build_program(repeat: int = 1):
    nc = bacc.Bacc("TRN2", target_bir_lowering=False, debug=False, num_devices=8)
    d = {
        "xbf": nc.declare_dram_parameter("xbf", [C, N], BF16, isOutput=False),
        "w8": nc.declare_dram_parameter("w8", [128, 2048], U8, isOutput=False),
        "smalls": nc.declare_dram_parameter("smalls", [128, 8], F32, isOutput=False),
        "gm": nc.declare_dram_parameter("gm", [128, 128], F32, isOutput=False),
        "i16": nc.declare_dram_parameter("i16", [128, 128], BF16, isOutput=False),
        "out": nc.declare_dram_parameter("out", [C, N], F32, isOutput=True),
    }
    with tile.TileContext(nc) as tc:
        for _ in range(repeat):
            _emit(nc, tc, d)
    nc.compile()
    return nc


def make_in_maps(x, norm_w, norm_b, qkv_w, qkv_b, proj_w, proj_b):
    x = np.asarray(x, np.float32)
    B = x.shape[0]
    qkv_w = np.asarray(qkv_w, np.float32)
    qkv_b = np.asarray(qkv_b, np.float32)
    proj_w = np.asarray(proj_w, np.float32)
    proj_b = np.asarray(proj_b, np.float32)
    FP8NP = ml_dtypes.float8_e4m3

    qkvT = (16.0 * qkv_w).T                      # [256, 768]
    projT = (16.0 * proj_w).T                    # [256, 256]
    wall = np.concatenate([qkvT, projT], axis=1)  # [256, 1024]
    w8 = wall.reshape(2, 128, 1024).transpose(1, 0, 2)  # [p, r, d]
    w8 = np.ascontiguousarray(w8.astype(FP8NP).view(np.uint8).reshape(128, 2048))

    projb_f = proj_b + proj_w @ qkv_b[2 * C:]    # fold v-bias into proj bias
    smalls = np.zeros((128, 8), np.float32)
    smalls[:, 0:2] = (16.0 * qkv_b[:C]).reshape(2, 128).T
    smalls[:, 2:4] = projb_f.reshape(2, 128).T
    smalls[:, 4:6] = np.asarray(norm_w, np.float32).reshape(2, 128).T
    smalls[:, 6:8] = np.asarray(norm_b, np.float32).reshape(2, 128).T
    shared = {
        "w8": w8,
        "smalls": smalls,
        "gm": (np.arange(128)[:, None] // 8 == np.arange(128)[None, :] // 8).astype(np.float32),
        "i16": (16.0 * np.eye(128, dtype=np.float32)).astype(ml_dtypes.bfloat16),
    }
    return [
        dict(shared,
             xbf=np.ascontiguousarray(x[b].reshape(C, N)).astype(ml_dtypes.bfloat16))
        for b in range(B)
    ]


_NC_CACHE = {}


def get_program(repeat: int = 1):
    if repeat not in _NC_CACHE:
        _NC_CACHE[repeat] = build_program(repeat)
    return _NC_CACHE[repeat]


def kernel(x, norm_w, norm_b, qkv_w, qkv_b, proj_w, proj_b):
    x = np.asarray(x, np.float32)
    B, C_, H_, W_ = x.shape
    in_maps = make_in_maps(x, norm_w, norm_b, qkv_w, qkv_b, proj_w, proj_b)
    nc = get_program()
    res = run_bass_kernel_spmd(nc, in_maps, core_ids=list(range(len(in_maps))))
    out = np.stack([np.asarray(res.results[b]["out"], np.float32) for b in range(B)])
    return out.reshape(B, C_, H_, W_)


# revision 32
# speedup vs baseline: 1.9191x; 1.0132x over previous
"""Trainium2 Bass kernel for nn_AttentionBlock (GroupNorm + spatial self-attention
+ residual). Full inputs in, full outputs out; data-parallel over batch (B=8)
across 8 NeuronCores; each core processes one [C=256, N=4096] image.

Design (full-fp8 attention; see study.py for the precision budget):
  - All big matmuls run fp8-e4m3 DoubleRow (0.5 cycles/row, 4x less PE time
    than two bf16 chunk matmuls for a 256-contraction). Weights are scaled
    x16 on the host so their sigma~1 lands in fp8's normal range; the score
    scale absorbs 1/256 and the softmax denominator absorbs the v-side 16.
  - DR layouts are [128, 2, n] with channel = r*128 + p.
  - q = 16(q_mm + bq) fp8; k = 16 k_mm fp8 (k-bias provably cancels in the
    softmax over j); v = 16 v_mm fp8 transposed to [n, c] pairs (v-bias passes
    through softmax and is folded into proj_b on the host).
  - scores s' = k'^T q' = 256 s, PSUM f32; w = exp(s'*SCALE/256 - 4) in fp8:
    the e^-4 offset keeps fp8 in range and cancels in normalization. exp runs
    split across two engines: ACT (exact exp, fp8 out) for the first
    N_ACT_PAIRS j-pairs of each stripe, DVE for the rest via a Schraudolph
    bit-trick (uint8 = rne(8/ln2*u + 56) is the fp8-e4m3 encoding of e^u;
    DVE's f32->uint8 convert rounds+saturates, verified on HW).
  - softmax denominator: DR ones-matmul (value 16 = v descale) accumulating
    [1, SW] in PSUM; reciprocal_approx_fast on DVE; partition_broadcast on
    the (otherwise idle) Pool engine; a = a_ps * rinv -> fp8 pairs (DVE).
  - proj fp8 DR with the residual accumulated into the same PSUM group via a
    16*I bf16 identity-matmul of xbf; o = Identity(o_ps/16 + projb') on ACT
    writes the final output, DMA'd on the SP HWDGE queue.
  - PSUM: 5 per-jt score banks + 2 AV banks + 1 bank shared by the
    denominator and o_ps (temporally disjoint, same pool tag). PE writes into
    a shared bank via a second aliased matmul fail BIR verification, so the
    rinv broadcast stays on Pool rather than a PE rank-1 matmul.
"""

import sys

try:
    import concourse  # noqa: F401
except ImportError:
    sys.path.insert(0, "/opt/trn_rl_repo")

import numpy as np
import ml_dtypes

import bass_rust as _bass_rust
import concourse.bacc as bacc
import concourse.tile as tile
from concourse import mybir
from concourse import bass_isa
from concourse.bass_utils import run_bass_kernel_spmd

F32 = mybir.dt.float32
BF16 = mybir.dt.bfloat16
FP8 = mybir.dt.float8e4
U8 = mybir.dt.uint8
AF = mybir.ActivationFunctionType
ALU = mybir.AluOpType
AX = mybir.AxisListType
DR = mybir.MatmulPerfMode.DoubleRow

C = 256          # channels
N = 4096         # spatial positions
GROUPS = 32
EPS = 1e-5
SCALE = C ** -0.5
SC2 = SCALE / 256.0          # fold the x16 q/k weight scaling out of scores
OFFSET = -4.0                # exp offset; cancels in softmax normalization
SCHRA_A = 8.0 / np.log(2.0)  # fp8-e4m3 schraudolph slope
NSTRIPE = 8
SW = N // NSTRIPE            # 512
NPAIR = N // 256             # 16 j-pairs (j-tiles of 128, two per pair)
PLAG = 6                     # pairs of lag between exp production and AV/ones
NJT = N // 128               # 32 j-tiles per stripe
# exp engine per j-tile, spread so ACT (faster, ~18 tiles) and DVE (~14) run
# concurrently within a stripe (a contiguous split serializes them through
# the score-buffer WAR).
ACT_JTS = frozenset(j for j in range(NJT) if (j * 18) // NJT != ((j + 1) * 18) // NJT)
GSIZE = (C // GROUPS) * N


def _emit(nc, tc, d):
    const = tc.alloc_tile_pool(name="const", bufs=1)

    # --- input DMAs: bf16 x split across both HWDGE queues, then weights ---
    xbf = []
    for t in range(2):
        xb_ = const.tile([128, N], BF16, tag=f"xbf{t}", name=f"xbf{t}")
        for g, eng in enumerate((nc.sync, nc.scalar)):
            eng.dma_start(xb_[:, g * (N // 2):(g + 1) * (N // 2)],
                          d["xbf"][t * 128:(t + 1) * 128,
                                   g * (N // 2):(g + 1) * (N // 2)])
        xbf.append(xb_)

    w8 = const.tile([128, 2, 1024], FP8, tag="w8")
    nc.scalar.dma_start(w8[:].bitcast(U8), d["w8"][:])
    wq8 = w8[:, :, 0:768]     # q 0:256, k 256:512, v 512:768 (x16 scaled)
    wp8 = w8[:, :, 768:1024]  # proj (x16 scaled)
    smalls = const.tile([128, 8], F32, tag="smalls")
    nc.scalar.dma_start(smalls[:], d["smalls"][:])
    qb16 = smalls[:, 0:2]
    projb = smalls[:, 2:4]
    nw = smalls[:, 4:6]
    nb = smalls[:, 6:8]
    gm = const.tile([128, 128], F32, tag="gm")
    nc.scalar.dma_start(gm[:], d["gm"][:])

    i16 = const.tile([128, 128], BF16, tag="i16")
    nc.scalar.dma_start(i16[:], d["i16"][:])
    ones16 = const.tile([128, 2, 16], FP8, tag="ones16")
    nc.vector.memset(ones16[:], 16.0)
    negoff = const.tile([128, 1], F32, tag="negoff")
    nc.vector.memset(negoff[:], OFFSET)
    onecol = const.tile([1, 128], F32, tag="onecol")
    nc.vector.memset(onecol[:], 1.0)

    # --- phase A: groupnorm stats -> per-chunk scale/bias ---
    stats = const.tile([128, 4], F32, tag="stats")
    scl = const.tile([128, 2], F32, tag="scl")
    bia = const.tile([128, 2], F32, tag="bia")
    gstats_mm = None
    with tc.tile_pool(name="scratch", bufs=2) as scr, \
         tc.tile_pool(name="pa_ps", bufs=1, space="PSUM") as pa_ps:
        for t in range(2):
            nc.vector.reduce_sum(stats[:, 2 * t:2 * t + 1], xbf[t][:], axis=AX.X)
            sq = scr.tile([128, N], F32, tag="sq")
            nc.scalar.activation(sq[:], xbf[t][:], AF.Square,
                                 accum_out=stats[:, 2 * t + 1:2 * t + 2])
            gstats = pa_ps.tile([128, 2], F32, tag=f"gstats{t}", name=f"gstats{t}")
            gstats_mm = nc.tensor.matmul(gstats[:], gm[:], stats[:, 2 * t:2 * t + 2],
                                         start=True, stop=True)
            mex = const.tile([128, 2], F32, tag=f"mex{t}", name=f"mex{t}")
            nc.vector.tensor_scalar_mul(mex[:], gstats[:], 1.0 / GSIZE)
            mean = mex[:, 0:1]
            ex2 = mex[:, 1:2]
            var = const.tile([128, 1], F32, tag=f"var{t}", name=f"var{t}")
            std = const.tile([128, 1], F32, tag=f"std{t}", name=f"std{t}")
            rstd = const.tile([128, 1], F32, tag=f"rstd{t}", name=f"rstd{t}")
            negm2 = const.tile([128, 1], F32, tag=f"negm2{t}", name=f"negm2{t}")
            nc.vector.scalar_tensor_tensor(negm2[:], mean, -1.0, mean,
                                           op0=ALU.mult, op1=ALU.mult)
            nc.vector.scalar_tensor_tensor(var[:], ex2, EPS, negm2[:],
                                           op0=ALU.add, op1=ALU.add)
            nc.scalar.activation(std[:], var[:], AF.Sqrt)
            nc.vector.reciprocal(rstd[:], std[:])
            nc.vector.tensor_mul(scl[:, t:t + 1], nw[:, t:t + 1], rstd[:])
            mscl = const.tile([128, 1], F32, tag=f"mscl{t}", name=f"mscl{t}")
            nc.vector.tensor_mul(mscl[:], mean, scl[:, t:t + 1])
            nc.vector.tensor_sub(bia[:, t:t + 1], nb[:, t:t + 1], mscl[:])
        # preload the exp table set while phase B runs (one-time ~2.7us)
        warm = const.tile([128, 1], F32, tag="expwarm")
        nc.scalar.activation(warm[:], stats[:, 0:1], AF.Exp, scale=0.0)

    # --- phase B: h (fp8 DR), q/k fp8 DR per-stripe tiles, vT fp8 pairs ---
    # Per-stripe q/k tiles keep the dependency granularity fine enough that
    # phase C's first score matmuls start as soon as phase B's first stripe
    # lands, instead of waiting for all of q/k.
    qt = [const.tile([128, 2, SW], FP8, tag=f"qt{s}", name=f"qt{s}")
          for s in range(NSTRIPE)]
    kt = [const.tile([128, 2, SW], FP8, tag=f"kt{s}", name=f"kt{s}")
          for s in range(NSTRIPE)]
    vtp = []
    for jp in range(NPAIR):
        vtp.append(const.tile([128, 2, 256], FP8, tag=f"vtp{jp}", name=f"vtp{jp}"))

    with tc.tile_pool(name="hpool", bufs=3) as hp, \
         tc.tile_pool(name="pbk_ps", bufs=2, space="PSUM") as pbk, \
         tc.tile_pool(name="pbq_ps", bufs=2, space="PSUM") as pbq, \
         tc.tile_pool(name="pbv_ps", bufs=2, space="PSUM") as pbv:
        for s in range(NSTRIPE):
            sl = slice(s * SW, (s + 1) * SW)
            ht = hp.tile([128, 2, SW], FP8, tag="h", name="h")
            for t in range(2):
                nc.scalar.activation(ht[:, t, :], xbf[t][:, sl], AF.Identity,
                                     bias=bia[:, t:t + 1], scale=scl[:, t:t + 1])
            # k (no bias: it cancels in the softmax over j)
            kps = pbk.tile([128, 1024], F32, tag="kps", name="kps")
            for t in range(2):
                nc.tensor.matmul(kps[:, t * SW:(t + 1) * SW],
                                 wq8[:, :, 256 + t * 128:256 + (t + 1) * 128],
                                 ht[:], start=True, stop=True, perf_mode=DR)
            nc.scalar.activation(kt[s][:, :, :], kps[:], AF.Copy)
            # q (+16*bias)
            for t in range(2):
                qps = pbq.tile([128, SW], F32, tag="qps", name="qps")
                nc.tensor.matmul(qps[:], wq8[:, :, t * 128:(t + 1) * 128],
                                 ht[:], start=True, stop=True, perf_mode=DR)
                nc.vector.tensor_scalar_add(qt[s][:, t, :], qps[:], qb16[:, t:t + 1])
            # vT pairs: two n4-chunks of 128 -> one [128, 512] psum -> one copy
            for half in range(2):
                vps = pbv.tile([128, 512], F32, tag="vps", name="vps")
                for par in range(2):
                    n4 = half * 2 + par
                    nc.tensor.matmul(vps[:, par * 256:(par + 1) * 256],
                                     ht[:, :, n4 * 128:(n4 + 1) * 128],
                                     wq8[:, :, 512:768], start=True, stop=True,
                                     perf_mode=DR)
                jp = s * 2 + half
                nc.vector.tensor_copy(vtp[jp][:, :, :], vps[:])

    # --- phase C: attention + proj + residual, per i-stripe ---
    # PSUM: s_ps 5 banks + a_ps 2 + do_ps 1 (denominator and o_ps share one
    # bank; their lifetimes are disjoint within the stripe-tail rotation).
    with tc.tile_pool(name="wpool", bufs=PLAG + 3) as wpo, \
         tc.tile_pool(name="misc", bufs=3) as mp, \
         tc.tile_pool(name="s_ps", bufs=5, space="PSUM") as spo, \
         tc.tile_pool(name="a_ps", bufs=2, space="PSUM") as apo, \
         tc.tile_pool(name="do_ps", bufs=1, space="PSUM") as dpo:
        opo = dpo

        def make_tail(ist, denom, a_ps):
            sl = slice(ist * SW, (ist + 1) * SW)
            st = {}

            def part1():
                rinv = mp.tile([1, SW], F32, tag="rinv")
                nc.vector.reciprocal_approx_fast(out=rinv[:], in_=denom[:])
                # broadcast rinv to all partitions with a rank-1 f32 matmul
                # into the shared do-bank (its lifetime fits between the
                # denominator read and the o_ps writes)
                rb = mp.tile([128, SW], F32, tag="rb")
                nc.gpsimd.partition_broadcast(rb[:], rinv[:], channels=128)
                st["rb"] = rb

            def part2():
                a8 = mp.tile([128, 2, SW], FP8, tag="a8")
                for ct in range(2):
                    nc.vector.tensor_mul(a8[:, ct, :], a_ps[ct][:], st["rb"][:])
                st["a8"] = a8

            def part3():
                a8 = st["a8"]
                for dt in range(2):
                    o_ps = opo.tile([128, SW], F32, tag="do", name="ops")
                    nc.tensor.matmul(o_ps[:], wp8[:, :, dt * 128:(dt + 1) * 128],
                                     a8[:], start=True, stop=False, perf_mode=DR)
                    # residual: o_ps += 16*xbf via identity matmul (x16 so the
                    # 1/16 proj descale below leaves x unscaled)
                    nc.tensor.matmul(o_ps[:], i16[:], xbf[dt][:, sl],
                                     start=False, stop=True)
                    o_t = mp.tile([128, SW], F32, tag=f"ot{dt}", name=f"ot{dt}")
                    nc.scalar.activation(o_t[:], o_ps[:], AF.Identity,
                                         bias=projb[:, dt:dt + 1], scale=1.0 / 16.0)
                    nc.sync.dma_start(d["out"][dt * 128:(dt + 1) * 128, sl], o_t[:])

            return [part1, part2, part3]

        pending = None
        for ist in range(NSTRIPE):
            isl = slice(ist * SW, (ist + 1) * SW)
            denom = None
            a_ps = None
            wpairs = {}
            for p in range(NPAIR + PLAG):
                if p < NPAIR:
                    wt = wpo.tile([128, 2, SW], FP8, tag="wp", name="wp")
                    for r in range(2):
                        jt = 2 * p + r
                        s_t = spo.tile([128, SW], F32, tag="sps", name="sps")
                        nc.tensor.matmul(s_t[:],
                                         kt[jt // 4][:, :, (jt % 4) * 128:(jt % 4 + 1) * 128],
                                         qt[ist][:], start=True, stop=True,
                                         perf_mode=DR)
                        if jt in ACT_JTS:
                            nc.scalar.activation(wt[:, r, :], s_t[:], AF.Exp,
                                                 bias=negoff[:], scale=SC2)
                        else:
                            nc.vector.tensor_scalar(
                                wt[:, r, :].bitcast(U8), s_t[:],
                                SCHRA_A * SC2, 56.0 + OFFSET * SCHRA_A,
                                op0=ALU.mult, op1=ALU.add)
                    wpairs[p] = wt
                if pending is not None:
                    if p == 0:
                        pending[0]()
                    elif p == 1:
                        pending[1]()
                    elif p == 2:
                        pending[2]()
                        pending = None
                if p == 3:
                    # allocated here (not at stripe start) so the shared
                    # den/o_ps bank's rotation order matches temporal order
                    denom = dpo.tile([1, SW], F32, tag="do", name="den")
                if p >= PLAG:
                    p2 = p - PLAG
                    w2 = wpairs.pop(p2)
                    if a_ps is None:
                        a_ps = [apo.tile([128, SW], F32, tag="aps", name="aps")
                                for _ in range(2)]
                    nc.tensor.matmul(denom[:], ones16[:, :, 0:1], w2[:],
                                     start=(p2 == 0), stop=(p2 == NPAIR - 1),
                                     perf_mode=DR)
                    for ct in range(2):
                        nc.tensor.matmul(a_ps[ct][:],
                                         vtp[p2][:, :, ct * 128:(ct + 1) * 128],
                                         w2[:], start=(p2 == 0),
                                         stop=(p2 == NPAIR - 1), perf_mode=DR)
            pending = make_tail(ist, denom, a_ps)
        for part in pending:
            part()

    const.release()


def build_program(repeat: int = 1):
    nc = bacc.Bacc("TRN2", target_bir_lowering=False, debug=False, num_devices=8)
    d = {
        "xbf": nc.declare_dram_parameter("xbf", [C, N], BF16, isOutput=False),
        "w8": nc.declare_dram_parameter("w8", [128, 2048], U8, isOutput=False),
        "smalls": nc.declare_dram_parameter("smalls", [128, 8], F32, isOutput=False),
        "gm": nc.declare_dram_parameter("gm", [128, 128], F32, isOutput=False),
        "i16": nc.declare_dram_parameter("i16", [128, 128], BF16, isOutput=False),
        "out": nc.declare_dram_parameter("out", [C, N], F32, isOutput=True),
    }
    with tile.TileContext(nc) as tc:
        for _ in range(repeat):
            _emit(nc, tc, d)
    nc.compile()
    return nc


def make_in_maps(x, norm_w, norm_b, qkv_w, qkv_b, proj_w, proj_b):
    x = np.asarray(x, np.float32)
    B = x.shape[0]
    qkv_w = np.asarray(qkv_w, np.float32)
    qkv_b = np.asarray(qkv_b, np.float32)
    proj_w = np.asarray(proj_w, np.float32)
    proj_b = np.asarray(proj_b, np.float32)
    FP8NP = ml_dtypes.float8_e4m3

    qkvT = (16.0 * qkv_w).T                      # [256, 768]
    projT = (16.0 * proj_w).T                    # [256, 256]
    wall = np.concatenate([qkvT, projT], axis=1)  # [256, 1024]
    w8 = wall.reshape(2, 128, 1024).transpose(1, 0, 2)  # [p, r, d]
    w8 = np.ascontiguousarray(w8.astype(FP8NP).view(np.uint8).reshape(128, 2048))

    projb_f = proj_b + proj_w @ qkv_b[2 * C:]    # fold v-bias into proj bias
    smalls = np.zeros((128, 8), np.float32)
    smalls[:, 0:2] = (16.0 * qkv_b[:C]).reshape(2, 128).T
    smalls[:, 2:4] = projb_f.reshape(2, 128).T
    smalls[:, 4:6] = np.asarray(norm_w, np.float32).reshape(2, 128).T
    smalls[:, 6:8] = np.asarray(norm_b, np.float32).reshape(2, 128).T
    shared = {
        "w8": w8,
        "smalls": smalls,
        "gm": (np.arange(128)[:, None] // 8 == np.arange(128)[None, :] // 8).astype(np.float32),
        "i16": (16.0 * np.eye(128, dtype=np.float32)).astype(ml_dtypes.bfloat16),
    }
    return [
        dict(shared,
             xbf=np.ascontiguousarray(x[b].reshape(C, N)).astype(ml_dtypes.bfloat16))
        for b in range(B)
    ]


_NC_CACHE = {}


def get_program(repeat: int = 1):
    if repeat not in _NC_CACHE:
        _NC_CACHE[repeat] = build_program(repeat)
    return _NC_CACHE[repeat]


def kernel(x, norm_w, norm_b, qkv_w, qkv_b, proj_w, proj_b):
    x = np.asarray(x, np.float32)
    B, C_, H_, W_ = x.shape
    in_maps = make_in_maps(x, norm_w, norm_b, qkv_w, qkv_b, proj_w, proj_b)
    nc = get_program()
    res = run_bass_kernel_spmd(nc, in_maps, core_ids=list(range(len(in_maps))))
    out = np.stack([np.asarray(res.results[b]["out"], np.float32) for b in range(B)])
    return out.reshape(B, C_, H_, W_)
